# revision 29
# baseline (speedup 1.0000x reference)
"""MLA (low-rank QKV projection + GQA attention) Bass kernel for 8 trn2 cores.

Problem shapes (hardcoded):
  x [B=2, T=2048, D=2048], Wq1 [512,2048], Wq2 [2048,512],
  Wk1/Wv1 [256,2048], Wk2/Wv2 [512,256], Wo [2048,2048]
  HQ=16 q-heads, HKV=4 kv-heads (GROUP=4), DH=128.

Sharding: core c = (b, g) with b = c//4 (data-parallel over batch),
g = c%4 (tensor-parallel over head groups). Each core owns q-heads
{4g..4g+3} and kv-head g for its batch's 2048 tokens. Host folds
(W2_head @ W1) into per-head direct projections (exact math in float64),
pre-transposes x[b] to [D, T], and sums the 4 per-core partial Wo
outputs per batch.

Device per core:
  phase 1: qT [128, 4*2048], kT [128, 2048], vT [128, 2048] via folded
           weights; vT PE-transposed into v (key tokens on partitions),
           interleaved with the projection loop.
  phase 2: per (qc, h) chunk of 512 queries: scores^T = kT.T @ qT chunk,
           E = exp(scale*S) (ACT), PV accumulate + ones-row sumexp
           matmul, store unnormalized attnT (bf16), sumexp rows.
           Chunks are software-pipelined (next chunk's scores interleave
           with this chunk's PV) and ordered qc-major so phases 3/4 for
           qc can start while qc+1 is still in attention.
  phase 3: per qc: PE-transpose sumexp rows [4,128] -> [128,4], vector
           reciprocal -> rT (per-token-per-head normalizers).
  phase 4: per (token tile, dmodel chunk): 4 per-head Wo matmuls, then a
           fused scale-and-accumulate chain spread over scalar/vector/
           gpsimd engines, normalized output DMA'd to DRAM.
"""

import os
import numpy as np

import concourse.bass as bass
import concourse.tile as tile
from concourse import mybir
from concourse import bass_utils

D_MODEL, HQ, HKV, RQ, RKV = 2048, 16, 4, 512, 256
DH = D_MODEL // HQ            # 128
GROUP = HQ // HKV             # 4
B, T = 2, 2048
NCORES = 8
NGROUP = 4                    # tensor-parallel groups (one per kv head)
HPC = HQ // NGROUP            # 4 q-heads per core
SCALE = 1.0 / np.sqrt(DH)

NK = D_MODEL // 128           # 16 contraction tiles over D
NTC = T // 512                # 4 token chunks of 512
NKK = T // 128                # 16 key tiles of 128
NQC = T // 512                # 4 query chunks of 512
NTT = T // 128                # 16 token tiles of 128

f32 = mybir.dt.float32


class _TC(tile.TileContext):
    pass


_nop_ctr = [0]


def _split_multi_waits(nc):
    """This walrus build's CoreV3 lowering accepts only ONE sync-wait per
    instruction; move extra waits onto same-engine single-wait nops inserted
    immediately before the instruction."""
    for f in nc.m.functions:
        for bb in f.blocks:
            insts = list(bb.instructions)
            out = []
            changed = False
            for ins in insts:
                si = ins.sync_info
                if si is not None and si.on_wait and len(si.on_wait) > 1:
                    waits = list(si.on_wait)
                    for w in waits[:-1]:
                        _nop_ctr[0] += 1
                        nop = mybir.InstNoOp(
                            name=f"waitsplit_{_nop_ctr[0]}",
                            ins=[],
                            outs=[],
                            engine=ins.engine,
                        )
                        nop.sync_info = mybir.SyncInfo(on_wait=[w], on_update=[])
                        nc.register_instruction(nop)
                        out.append(nop)
                    ins.sync_info = mybir.SyncInfo(
                        on_wait=[waits[-1]], on_update=list(si.on_update)
                    )
                    changed = True
                out.append(ins)
            if changed:
                bb.instructions = out


def _build(mmdt, use_mask):
    nc = bass.Bass(trn_type="TRN2")
    xT = nc.dram_tensor("xT", (D_MODEL, T), mmdt, kind="ExternalInput")
    wq = nc.dram_tensor("wq", (D_MODEL, HPC * DH), mmdt, kind="ExternalInput")
    wk = nc.dram_tensor("wk", (D_MODEL, DH), mmdt, kind="ExternalInput")
    wv = nc.dram_tensor("wv", (D_MODEL, DH), mmdt, kind="ExternalInput")
    woT = nc.dram_tensor("woT", (HPC * DH, D_MODEL), mmdt, kind="ExternalInput")
    identm = nc.dram_tensor("identm", (128, 128), mmdt, kind="ExternalInput")
    identf = nc.dram_tensor("identf", (128, 128), f32, kind="ExternalInput")
    if use_mask:
        # pre-transposed, pre-scaled by sqrt(DH): [k, q]
        maskT = nc.dram_tensor("maskT", (T, T), f32, kind="ExternalInput")
    else:
        maskT = None
    out = nc.dram_tensor("out", (T, D_MODEL), f32, kind="ExternalOutput")

    Exp = mybir.ActivationFunctionType.Exp
    Copy = mybir.ActivationFunctionType.Copy
    Mult = mybir.AluOpType.mult
    Add = mybir.AluOpType.add

    with _TC(nc) as tc:
        with (
            tc.tile_pool(name="persist", bufs=1) as persist,
            tc.tile_pool(name="consts", bufs=1) as consts,
        ):
            qT_s = persist.tile([128, HPC * T], mmdt)     # head h at cols h*T
            kT_s = persist.tile([128, T], mmdt)
            vT_s = persist.tile([128, T], mmdt)
            v_s = persist.tile([128, T], mmdt)            # kk-tile t at cols t*128
            attnT_s = persist.tile([128, HPC * T], mmdt)  # unnormalized PV
            sumexp_s = persist.tile([128, T], f32)        # head h on partition 32*h
            rT_s = persist.tile([128, NTT * HPC], f32)    # recip, tok on partition
            woT_s = persist.tile([128, HPC * D_MODEL], mmdt)
            onesf_s = consts.tile([128, 1], f32)
            onesb_s = consts.tile([128, 1], mmdt)
            identm_s = consts.tile([128, 128], mmdt)
            identf_s = consts.tile([128, 128], f32)
            nc.vector.memset(onesf_s[:], 1.0)
            nc.vector.memset(onesb_s[:], 1.0)
            # unused partitions of sumexp_s flow through the phase-3
            # transpose; init so no garbage/non-finite values are read
            nc.vector.memset(sumexp_s[:], 1.0)

            # ---------------- phase 1: QKV projections ----------------
            # Weight/const DMA goes on the (otherwise idle) gpsimd queue;
            # sync queue carries x tiles. Per-kd slices so the first
            # matmuls can start early. Two passes (k/v then q) so every
            # psum tag is double-buffered within the 8-bank budget; x is
            # simply re-DMA'd for the second pass.
            with (
                tc.tile_pool(name="wgt", bufs=1) as wgt,
                tc.tile_pool(name="xin", bufs=4) as xin,
            ):
                wq_s = wgt.tile([128, NK * HPC * DH], mmdt)
                wk_s = wgt.tile([128, NK * DH], mmdt)
                wv_s = wgt.tile([128, NK * DH], mmdt)
                # whole-tensor rearranged DMAs: one descriptor stream per
                # weight, so PE is never paced by per-slice issue cost
                nc.gpsimd.dma_start(
                    wk_s[:].rearrange("p (t m) -> p t m", t=NK),
                    wk[:].rearrange("(t p) m -> p t m", p=128),
                )
                nc.gpsimd.dma_start(
                    wv_s[:].rearrange("p (t m) -> p t m", t=NK),
                    wv[:].rearrange("(t p) m -> p t m", p=128),
                )
                nc.gpsimd.dma_start(identm_s[:], identm[:])
                nc.gpsimd.dma_start(identf_s[:], identf[:])
                nc.scalar.dma_start(
                    wq_s[:].rearrange("p (t m) -> p t m", t=NK),
                    wq[:].rearrange("(t p) m -> p t m", p=128),
                )
                for h in range(HPC):
                    nc.scalar.dma_start(
                        woT_s[:, h * D_MODEL : (h + 1) * D_MODEL],
                        woT[h * 128 : (h + 1) * 128, :],
                    )

                # pass A: k and v projections, v transposed as chunks finish
                with (
                    tc.tile_pool(name="kvp", bufs=2, space="PSUM") as kvp,
                    tc.tile_pool(name="trp", bufs=2, space="PSUM") as trp,
                ):
                    def emit_vtr(n):
                        for t in range(n * 4, n * 4 + 4):
                            tr = trp.tile([128, 128], mmdt, tag="tr", name=f"tr_{t}")
                            nc.tensor.transpose(
                                tr[:], vT_s[:, t * 128 : (t + 1) * 128], identm_s[:]
                            )
                            nc.vector.tensor_copy(
                                v_s[:, t * 128 : (t + 1) * 128], tr[:]
                            )

                    for n in range(NTC):
                        nsl = slice(n * 512, (n + 1) * 512)
                        ps_k = kvp.tile([128, 512], f32, tag="psk", name=f"psk_{n}")
                        ps_v = kvp.tile([128, 512], f32, tag="psv", name=f"psv_{n}")
                        for kd in range(NK):
                            xt = xin.tile([128, 512], mmdt, tag="xt", name=f"xta_{n}_{kd}")
                            nc.sync.dma_start(
                                xt[:], xT[kd * 128 : (kd + 1) * 128, nsl]
                            )
                            st, sp = kd == 0, kd == NK - 1
                            nc.tensor.matmul(
                                ps_k[:], wk_s[:, kd * 128 : (kd + 1) * 128], xt[:],
                                start=st, stop=sp,
                            )
                            nc.tensor.matmul(
                                ps_v[:], wv_s[:, kd * 128 : (kd + 1) * 128], xt[:],
                                start=st, stop=sp,
                            )
                        nc.scalar.activation(kT_s[:, nsl], ps_k[:], Copy)
                        nc.vector.tensor_copy(vT_s[:, nsl], ps_v[:])
                        if n > 0:
                            emit_vtr(n - 1)
                    emit_vtr(NTC - 1)

                # pass B: q projections, all four heads double-buffered
                with tc.tile_pool(name="qp", bufs=2, space="PSUM") as qp:
                    for n in range(NTC):
                        nsl = slice(n * 512, (n + 1) * 512)
                        ps_q = [
                            qp.tile([128, 512], f32, tag=f"psq{j}", name=f"psq{j}_{n}")
                            for j in range(HPC)
                        ]
                        for kd in range(NK):
                            xt = xin.tile([128, 512], mmdt, tag="xt", name=f"xtb_{n}_{kd}")
                            nc.sync.dma_start(
                                xt[:], xT[kd * 128 : (kd + 1) * 128, nsl]
                            )
                            st, sp = kd == 0, kd == NK - 1
                            for j in range(HPC):
                                nc.tensor.matmul(
                                    ps_q[j][:],
                                    wq_s[:, kd * 512 + j * 128 : kd * 512 + (j + 1) * 128],
                                    xt[:],
                                    start=st, stop=sp,
                                )
                        nc.scalar.activation(qT_s[:, 0 * T + n * 512 : 0 * T + (n + 1) * 512], ps_q[0][:], Copy)
                        nc.scalar.activation(qT_s[:, 1 * T + n * 512 : 1 * T + (n + 1) * 512], ps_q[1][:], Copy)
                        nc.vector.tensor_copy(qT_s[:, 2 * T + n * 512 : 2 * T + (n + 1) * 512], ps_q[2][:])
                        nc.vector.tensor_copy(qT_s[:, 3 * T + n * 512 : 3 * T + (n + 1) * 512], ps_q[3][:])

            # ---------------- phases 2+3+4 interleaved ----------------
            with (
                tc.tile_pool(name="epool", bufs=36) as epool,
                tc.tile_pool(name="esum", bufs=2) as esump,
                tc.tile_pool(name="omg", bufs=4) as omg,
                tc.tile_pool(name="anorm", bufs=6) as anorm,
                tc.tile_pool(name="mpool", bufs=3) as mpool,
                tc.tile_pool(name="stp", bufs=2, space="PSUM") as stp,
                tc.tile_pool(name="pvp", bufs=1, space="PSUM") as pvp,
                tc.tile_pool(name="sump", bufs=1, space="PSUM") as sump,
                tc.tile_pool(name="ntrp", bufs=2, space="PSUM") as ntrp,
                tc.tile_pool(name="wops", bufs=2, space="PSUM") as wops,
            ):
                chunks = [(qc, h) for qc in range(NQC) for h in range(HPC)]

                def emit_scores(ci, kt):
                    qc, h = chunks[ci]
                    qsl = qT_s[:, h * T + qc * 512 : h * T + (qc + 1) * 512]
                    ps_st = stp.tile([128, 512], f32, tag="st", name=f"st_{ci}_{kt}")
                    nc.tensor.matmul(
                        ps_st[:],
                        kT_s[:, kt * 128 : (kt + 1) * 128],
                        qsl,
                        start=True, stop=True,
                    )
                    if use_mask:
                        mt = mpool.tile([128, 512], f32, tag="mt", name=f"mt_{ci}_{kt}")
                        nc.sync.dma_start(
                            mt[:],
                            maskT[kt * 128 : (kt + 1) * 128, qc * 512 : (qc + 1) * 512],
                        )
                        nc.vector.tensor_add(ps_st[:], ps_st[:], mt[:])
                    e = epool.tile([128, 512], mmdt, tag="e", name=f"e_{ci}_{kt}")
                    nc.scalar.activation(e[:], ps_st[:], Exp, scale=SCALE)
                    return e

                aN_store = {}

                def emit_normA(h0, h1, it):
                    # stage 1 of in-place attnT normalization: transpose two
                    # head tiles -> tokens on partitions, scale by recip
                    # (per-partition). Stage 2 (normB) runs in a later drain
                    # slot so PE never waits on the vector scale.
                    for hh in (h0, h1):
                        asl = attnT_s[:, hh * T + it * 128 : hh * T + (it + 1) * 128]
                        tr1 = ntrp.tile([128, 128], mmdt, tag="ntr", name=f"tr1_{hh}_{it}")
                        nc.tensor.transpose(tr1[:], asl, identm_s[:])
                        aN = anorm.tile([128, 128], mmdt, tag="an", name=f"aN_{hh}_{it}")
                        nc.vector.tensor_scalar_mul(
                            aN[:], tr1[:], rT_s[:, it * HPC + hh : it * HPC + hh + 1]
                        )
                        aN_store[(hh, it)] = aN

                def emit_normB(h0, h1, it):
                    # stage 2: transpose the scaled tiles back into attnT
                    for hh in (h0, h1):
                        aN = aN_store.pop((hh, it))
                        tr2 = ntrp.tile([128, 128], mmdt, tag="ntr", name=f"tr2_{hh}_{it}")
                        nc.tensor.transpose(tr2[:], aN[:], identm_s[:])
                        nc.vector.tensor_copy(
                            attnT_s[:, hh * T + it * 128 : hh * T + (it + 1) * 128],
                            tr2[:],
                        )

                def emit_p4_item(it, dc, drain_eng):
                    # 4 per-head Wo matmuls accumulate into one psum bank
                    p = wops.tile([128, 512], f32, tag="wo", name=f"wo_{it}_{dc}")
                    for h in range(HPC):
                        nc.tensor.matmul(
                            p[:],
                            attnT_s[:, h * T + it * 128 : h * T + (it + 1) * 128],
                            woT_s[:, h * D_MODEL + dc * 512 : h * D_MODEL + (dc + 1) * 512],
                            start=(h == 0), stop=(h == HPC - 1),
                        )
                    oo = omg.tile([128, 512], f32, tag="oo", name=f"oo_{it}_{dc}")
                    if drain_eng == "scalar":
                        nc.scalar.activation(oo[:], p[:], Copy)
                    else:
                        nc.vector.tensor_copy(oo[:], p[:])
                    nc.sync.dma_start(
                        out[it * 128 : (it + 1) * 128, dc * 512 : (dc + 1) * 512],
                        oo[:],
                    )

                def drain_unit(u):
                    kind = u[0]
                    if kind == "normA":
                        emit_normA(u[1], u[2], u[3])
                    elif kind == "normB":
                        emit_normB(u[1], u[2], u[3])
                    else:
                        emit_p4_item(u[1], u[2], u[3])

                p4q = []  # deferred norm/p4 work units
                es_cur = [emit_scores(0, kt) for kt in range(NKK)]
                for ci in range(len(chunks)):
                    qc, h = chunks[ci]
                    ps_pv = pvp.tile([128, 512], f32, tag="pv", name=f"pv_{ci}")
                    # sumexp: kt 0-5 summed on PE (ones-matmuls), kt 6-8 on
                    # gpsimd, kt 9-15 on vector; two folding matmuls at end.
                    ps_sum = sump.tile([1, 512], f32, tag="sum", name=f"sum_{ci}")
                    esumG = esump.tile([128, 512], f32, tag="esG", name=f"esG_{ci}")
                    esumV = esump.tile([128, 512], f32, tag="esV", name=f"esV_{ci}")
                    es_next = []
                    for kt in range(NKK):
                        st, sp = kt == 0, kt == NKK - 1
                        nc.tensor.matmul(
                            ps_pv[:],
                            v_s[:, kt * 128 : (kt + 1) * 128],
                            es_cur[kt][:],
                            start=st, stop=sp,
                        )
                        if kt < 4:
                            nc.tensor.matmul(
                                ps_sum[:], onesb_s[:], es_cur[kt][:],
                                start=st, stop=False,
                            )
                        elif kt < 9:
                            if kt == 4:
                                nc.gpsimd.tensor_copy(esumG[:], es_cur[kt][:])
                            else:
                                nc.gpsimd.tensor_add(esumG[:], esumG[:], es_cur[kt][:])
                        else:
                            if kt == 9:
                                nc.vector.tensor_copy(esumV[:], es_cur[kt][:])
                            else:
                                nc.vector.tensor_add(esumV[:], esumV[:], es_cur[kt][:])
                        if sp:
                            # free the pv bank as soon as accumulation stops
                            nc.vector.tensor_copy(
                                attnT_s[:, h * T + qc * 512 : h * T + (qc + 1) * 512],
                                ps_pv[:],
                            )
                        # defer the last 4 score emissions so the sumexp copy
                        # lands early in the scalar queue (frees the ps_sum
                        # bank before the next chunk's first ones-matmul)
                        if ci + 1 < len(chunks) and kt < 12:
                            es_next.append(emit_scores(ci + 1, kt))
                        if p4q and kt % 2 == 1:
                            drain_unit(p4q.pop(0))
                    # fold the gpsimd/vector partial sums into ps_sum
                    nc.tensor.matmul(
                        ps_sum[:], onesf_s[:], esumG[:], start=False, stop=False,
                    )
                    nc.tensor.matmul(
                        ps_sum[:], onesf_s[:], esumV[:], start=False, stop=True,
                    )
                    nc.scalar.activation(
                        sumexp_s[32 * h : 32 * h + 1, qc * 512 : (qc + 1) * 512],
                        ps_sum[0:1, :],
                        Copy,
                    )
                    if ci + 1 < len(chunks):
                        for kt in range(12, NKK):
                            es_next.append(emit_scores(ci + 1, kt))
                    es_cur = es_next
                    if h == HPC - 1:
                        # phase 3 for this qc: transpose sumexp rows, recip.
                        # Heads live at partitions {0,32,64,96} (engine
                        # partition-start constraint), so transpose the full
                        # 128 partitions and recip the 4 head columns.
                        for tt in range(4):
                            it = qc * 4 + tt
                            ps_r = ntrp.tile([128, 128], f32, tag="ntr", name=f"psr_{it}")
                            nc.tensor.transpose(
                                ps_r[:],
                                sumexp_s[:, qc * 512 + tt * 128 : qc * 512 + (tt + 1) * 128],
                                identf_s[:],
                            )
                            for hh in range(HPC):
                                nc.vector.reciprocal(
                                    rT_s[:, it * HPC + hh : it * HPC + hh + 1],
                                    ps_r[:, 32 * hh : 32 * hh + 1],
                                )
                        for tt in range(4):
                            it = qc * 4 + tt
                            p4q.append(("normA", 0, 1, it))
                            p4q.append(("normA", 2, 3, it))
                            p4q.append(("normB", 0, 1, it))
                            p4q.append(("normB", 2, 3, it))
                            for dc in range(4):
                                p4q.append(
                                    ("p4", it, dc, "scalar" if dc == 0 else "vector")
                                )
                for u in p4q:
                    drain_unit(u)

    _split_multi_waits(nc)
    return nc


_cache = {}


def _get_nc(mmdt_name, use_mask):
    key = (mmdt_name, use_mask)
    if key not in _cache:
        _cache[key] = _build(getattr(mybir.dt, mmdt_name), use_mask)
    return _cache[key]


def _np_dt(mmdt_name):
    if mmdt_name == "bfloat16":
        import ml_dtypes

        return ml_dtypes.bfloat16
    return np.float32


def _prep_inputs(x, attn_mask, Wq1, Wq2, Wk1, Wk2, Wv1, Wv2, Wo, mmdt_name):
    ndt = _np_dt(mmdt_name)
    identm = np.eye(128, dtype=np.float32).astype(ndt)
    identf = np.eye(128, dtype=np.float32)
    use_mask = bool(np.any(attn_mask))
    maskT = None
    if use_mask:
        maskT = np.ascontiguousarray(attn_mask[0, 0].T * np.sqrt(DH)).astype(
            np.float32
        )
    Wq1_64, Wq2_64 = Wq1.astype(np.float64), Wq2.astype(np.float64)
    Wk1_64, Wk2_64 = Wk1.astype(np.float64), Wk2.astype(np.float64)
    Wv1_64, Wv2_64 = Wv1.astype(np.float64), Wv2.astype(np.float64)
    xT_b = [
        np.ascontiguousarray(np.asarray(x[b]).T).astype(ndt) for b in range(B)
    ]
    in_maps = []
    for c in range(NCORES):
        b, g = divmod(c, NGROUP)
        h0 = g * HPC
        wq_f = (Wq2_64[h0 * DH : (h0 + HPC) * DH] @ Wq1_64).T  # [D, HPC*DH]
        wk_f = (Wk2_64[g * DH : (g + 1) * DH] @ Wk1_64).T      # [D, DH]
        wv_f = (Wv2_64[g * DH : (g + 1) * DH] @ Wv1_64).T
        woT_c = np.ascontiguousarray(Wo[:, h0 * DH : (h0 + HPC) * DH].T)
        m = {
            "xT": xT_b[b],
            "wq": np.ascontiguousarray(wq_f).astype(ndt),
            "wk": np.ascontiguousarray(wk_f).astype(ndt),
            "wv": np.ascontiguousarray(wv_f).astype(ndt),
            "woT": woT_c.astype(ndt),
            "identm": identm,
            "identf": identf,
        }
        if use_mask:
            m["maskT"] = maskT
        in_maps.append(m)
    return in_maps, use_mask


def run(x, attn_mask, Wq1, Wq2, Wk1, Wk2, Wv1, Wv2, Wo, **spmd_kwargs):
    mmdt_name = os.environ.get("BASS_MLA_DT", "bfloat16")
    in_maps, use_mask = _prep_inputs(
        x, attn_mask, Wq1, Wq2, Wk1, Wk2, Wv1, Wv2, Wo, mmdt_name
    )
    nc = _get_nc(mmdt_name, use_mask)
    res = bass_utils.run_bass_kernel_spmd(
        nc, in_maps, core_ids=list(range(NCORES)), **spmd_kwargs
    )
    out = np.zeros((B, T, D_MODEL), np.float64)
    for c in range(NCORES):
        out[c // NGROUP] += res.results[c]["out"]
    return out.astype(np.float32), res


def kernel(x, attn_mask, Wq1, Wq2, Wk1, Wk2, Wv1, Wv2, Wo):
    out, _ = run(x, attn_mask, Wq1, Wq2, Wk1, Wk2, Wv1, Wv2, Wo)
    return out


# revision 30
# speedup vs baseline: 1.0227x; 1.0227x over previous
"""MLA (low-rank QKV projection + GQA attention) Bass kernel for 8 trn2 cores.

Problem shapes (hardcoded):
  x [B=2, T=2048, D=2048], Wq1 [512,2048], Wq2 [2048,512],
  Wk1/Wv1 [256,2048], Wk2/Wv2 [512,256], Wo [2048,2048]
  HQ=16 q-heads, HKV=4 kv-heads (GROUP=4), DH=128.

Sharding: core c = (b, g) with b = c//4 (data-parallel over batch),
g = c%4 (tensor-parallel over head groups). Each core owns q-heads
{4g..4g+3} and kv-head g for its batch's 2048 tokens. Host folds
(W2_head @ W1) into per-head direct projections (exact math in float64),
pre-transposes x[b] to [D, T], and sums the 4 per-core partial Wo
outputs per batch.

Device per core:
  phase 1: qT [128, 4*2048], kT [128, 2048], vT [128, 2048] via folded
           weights; vT PE-transposed into v (key tokens on partitions),
           interleaved with the projection loop.
  phase 2: per (qc, h) chunk of 512 queries: scores^T = kT.T @ qT chunk,
           E = exp(scale*S) (ACT), PV accumulate + ones-row sumexp
           matmul, store unnormalized attnT (bf16), sumexp rows.
           Chunks are software-pipelined (next chunk's scores interleave
           with this chunk's PV) and ordered qc-major so phases 3/4 for
           qc can start while qc+1 is still in attention.
  phase 3: per qc: PE-transpose sumexp rows [4,128] -> [128,4], vector
           reciprocal -> rT (per-token-per-head normalizers).
  phase 4: per (token tile, dmodel chunk): 4 per-head Wo matmuls, then a
           fused scale-and-accumulate chain spread over scalar/vector/
           gpsimd engines, normalized output DMA'd to DRAM.
"""

import os
import numpy as np

import concourse.bass as bass
import concourse.tile as tile
from concourse import mybir
from concourse import bass_utils

D_MODEL, HQ, HKV, RQ, RKV = 2048, 16, 4, 512, 256
DH = D_MODEL // HQ            # 128
GROUP = HQ // HKV             # 4
B, T = 2, 2048
NCORES = 8
NGROUP = 4                    # tensor-parallel groups (one per kv head)
HPC = HQ // NGROUP            # 4 q-heads per core
SCALE = 1.0 / np.sqrt(DH)

NK = D_MODEL // 128           # 16 contraction tiles over D
NTC = T // 512                # 4 token chunks of 512
NKK = T // 128                # 16 key tiles of 128
NQC = T // 512                # 4 query chunks of 512
NTT = T // 128                # 16 token tiles of 128

f32 = mybir.dt.float32


class _TC(tile.TileContext):
    pass


_nop_ctr = [0]


def _split_multi_waits(nc):
    """This walrus build's CoreV3 lowering accepts only ONE sync-wait per
    instruction; move extra waits onto same-engine single-wait nops inserted
    immediately before the instruction."""
    for f in nc.m.functions:
        for bb in f.blocks:
            insts = list(bb.instructions)
            out = []
            changed = False
            for ins in insts:
                si = ins.sync_info
                if si is not None and si.on_wait and len(si.on_wait) > 1:
                    waits = list(si.on_wait)
                    for w in waits[:-1]:
                        _nop_ctr[0] += 1
                        nop = mybir.InstNoOp(
                            name=f"waitsplit_{_nop_ctr[0]}",
                            ins=[],
                            outs=[],
                            engine=ins.engine,
                        )
                        nop.sync_info = mybir.SyncInfo(on_wait=[w], on_update=[])
                        nc.register_instruction(nop)
                        out.append(nop)
                    ins.sync_info = mybir.SyncInfo(
                        on_wait=[waits[-1]], on_update=list(si.on_update)
                    )
                    changed = True
                out.append(ins)
            if changed:
                bb.instructions = out


def _build(mmdt, use_mask):
    nc = bass.Bass(trn_type="TRN2")
    xT = nc.dram_tensor("xT", (D_MODEL, T), mmdt, kind="ExternalInput")
    wq = nc.dram_tensor("wq", (D_MODEL, HPC * DH), mmdt, kind="ExternalInput")
    wk = nc.dram_tensor("wk", (D_MODEL, DH), mmdt, kind="ExternalInput")
    wv = nc.dram_tensor("wv", (D_MODEL, DH), mmdt, kind="ExternalInput")
    woT = nc.dram_tensor("woT", (HPC * DH, D_MODEL), mmdt, kind="ExternalInput")
    identm = nc.dram_tensor("identm", (128, 128), mmdt, kind="ExternalInput")
    identf = nc.dram_tensor("identf", (128, 128), f32, kind="ExternalInput")
    if use_mask:
        # pre-transposed, pre-scaled by sqrt(DH): [k, q]
        maskT = nc.dram_tensor("maskT", (T, T), f32, kind="ExternalInput")
    else:
        maskT = None
    out = nc.dram_tensor("out", (T, D_MODEL), f32, kind="ExternalOutput")

    Exp = mybir.ActivationFunctionType.Exp
    Copy = mybir.ActivationFunctionType.Copy
    Mult = mybir.AluOpType.mult
    Add = mybir.AluOpType.add

    with _TC(nc) as tc:
        with (
            tc.tile_pool(name="persist", bufs=1) as persist,
            tc.tile_pool(name="consts", bufs=1) as consts,
        ):
            qT_s = persist.tile([128, HPC * T], mmdt)     # head h at cols h*T
            kT_s = persist.tile([128, T], mmdt)
            vT_s = persist.tile([128, T], mmdt)
            v_s = persist.tile([128, T], mmdt)            # kk-tile t at cols t*128
            attnT_s = persist.tile([128, HPC * T], mmdt)  # unnormalized PV
            sumexp_s = persist.tile([128, T], f32)        # head h on partition 32*h
            rT_s = persist.tile([128, NTT * HPC], f32)    # recip, tok on partition
            woT_s = persist.tile([128, HPC * D_MODEL], mmdt)
            onesf_s = consts.tile([128, 1], f32)
            onesb_s = consts.tile([128, 1], mmdt)
            identm_s = consts.tile([128, 128], mmdt)
            identf_s = consts.tile([128, 128], f32)
            nc.vector.memset(onesf_s[:], 1.0)
            nc.vector.memset(onesb_s[:], 1.0)
            # unused partitions of sumexp_s flow through the phase-3
            # transpose; init so no garbage/non-finite values are read
            nc.vector.memset(sumexp_s[:], 1.0)

            # ---------------- phase 1: QKV projections ----------------
            # Weight/const DMA goes on the (otherwise idle) gpsimd queue;
            # sync queue carries x tiles. Per-kd slices so the first
            # matmuls can start early. Two passes (k/v then q) so every
            # psum tag is double-buffered within the 8-bank budget; x is
            # simply re-DMA'd for the second pass.
            with (
                tc.tile_pool(name="wgt", bufs=1) as wgt,
                tc.tile_pool(name="xin", bufs=4) as xin,
            ):
                wq_s = wgt.tile([128, NK * HPC * DH], mmdt)
                wk_s = wgt.tile([128, NK * DH], mmdt)
                wv_s = wgt.tile([128, NK * DH], mmdt)
                for kd in range(NK):
                    ksl = slice(kd * 128, (kd + 1) * 128)
                    nc.gpsimd.dma_start(
                        wk_s[:, kd * 128 : (kd + 1) * 128], wk[ksl, :]
                    )
                    nc.gpsimd.dma_start(
                        wv_s[:, kd * 128 : (kd + 1) * 128], wv[ksl, :]
                    )
                    if kd == 1:
                        nc.gpsimd.dma_start(identm_s[:], identm[:])
                        nc.gpsimd.dma_start(identf_s[:], identf[:])
                for kd in range(NK):
                    nc.scalar.dma_start(
                        wq_s[:, kd * 512 : (kd + 1) * 512],
                        wq[kd * 128 : (kd + 1) * 128, :],
                    )
                for h in range(HPC):
                    nc.scalar.dma_start(
                        woT_s[:, h * D_MODEL : (h + 1) * D_MODEL],
                        woT[h * 128 : (h + 1) * 128, :],
                    )

                # pass A: k and v projections, v transposed as chunks finish
                with (
                    tc.tile_pool(name="kvp", bufs=2, space="PSUM") as kvp,
                    tc.tile_pool(name="trp", bufs=2, space="PSUM") as trp,
                ):
                    def emit_vtr(n):
                        for t in range(n * 4, n * 4 + 4):
                            tr = trp.tile([128, 128], mmdt, tag="tr", name=f"tr_{t}")
                            nc.tensor.transpose(
                                tr[:], vT_s[:, t * 128 : (t + 1) * 128], identm_s[:]
                            )
                            nc.vector.tensor_copy(
                                v_s[:, t * 128 : (t + 1) * 128], tr[:]
                            )

                    for n in range(NTC):
                        nsl = slice(n * 512, (n + 1) * 512)
                        ps_k = kvp.tile([128, 512], f32, tag="psk", name=f"psk_{n}")
                        ps_v = kvp.tile([128, 512], f32, tag="psv", name=f"psv_{n}")
                        for kd in range(NK):
                            xt = xin.tile([128, 512], mmdt, tag="xt", name=f"xta_{n}_{kd}")
                            nc.sync.dma_start(
                                xt[:], xT[kd * 128 : (kd + 1) * 128, nsl]
                            )
                            st, sp = kd == 0, kd == NK - 1
                            nc.tensor.matmul(
                                ps_k[:], wk_s[:, kd * 128 : (kd + 1) * 128], xt[:],
                                start=st, stop=sp,
                            )
                            nc.tensor.matmul(
                                ps_v[:], wv_s[:, kd * 128 : (kd + 1) * 128], xt[:],
                                start=st, stop=sp,
                            )
                        nc.scalar.activation(kT_s[:, nsl], ps_k[:], Copy)
                        nc.vector.tensor_copy(vT_s[:, nsl], ps_v[:])
                        if n > 0:
                            emit_vtr(n - 1)
                    emit_vtr(NTC - 1)

                # pass B: q projections, all four heads double-buffered
                with tc.tile_pool(name="qp", bufs=2, space="PSUM") as qp:
                    for n in range(NTC):
                        nsl = slice(n * 512, (n + 1) * 512)
                        ps_q = [
                            qp.tile([128, 512], f32, tag=f"psq{j}", name=f"psq{j}_{n}")
                            for j in range(HPC)
                        ]
                        for kd in range(NK):
                            xt = xin.tile([128, 512], mmdt, tag="xt", name=f"xtb_{n}_{kd}")
                            nc.sync.dma_start(
                                xt[:], xT[kd * 128 : (kd + 1) * 128, nsl]
                            )
                            st, sp = kd == 0, kd == NK - 1
                            for j in range(HPC):
                                nc.tensor.matmul(
                                    ps_q[j][:],
                                    wq_s[:, kd * 512 + j * 128 : kd * 512 + (j + 1) * 128],
                                    xt[:],
                                    start=st, stop=sp,
                                )
                        nc.scalar.activation(qT_s[:, 0 * T + n * 512 : 0 * T + (n + 1) * 512], ps_q[0][:], Copy)
                        nc.scalar.activation(qT_s[:, 1 * T + n * 512 : 1 * T + (n + 1) * 512], ps_q[1][:], Copy)
                        nc.vector.tensor_copy(qT_s[:, 2 * T + n * 512 : 2 * T + (n + 1) * 512], ps_q[2][:])
                        nc.vector.tensor_copy(qT_s[:, 3 * T + n * 512 : 3 * T + (n + 1) * 512], ps_q[3][:])

            # ---------------- phases 2+3+4 interleaved ----------------
            with (
                tc.tile_pool(name="epool", bufs=36) as epool,
                tc.tile_pool(name="esum", bufs=2) as esump,
                tc.tile_pool(name="omg", bufs=4) as omg,
                tc.tile_pool(name="anorm", bufs=6) as anorm,
                tc.tile_pool(name="mpool", bufs=3) as mpool,
                tc.tile_pool(name="stp", bufs=3, space="PSUM") as stp,
                tc.tile_pool(name="pvp", bufs=1, space="PSUM") as pvp,
                tc.tile_pool(name="sump", bufs=1, space="PSUM") as sump,
                tc.tile_pool(name="ntrp", bufs=2, space="PSUM") as ntrp,
                tc.tile_pool(name="wops", bufs=1, space="PSUM") as wops,
            ):
                chunks = [(qc, h) for qc in range(NQC) for h in range(HPC)]

                def emit_scores(ci, kt):
                    qc, h = chunks[ci]
                    qsl = qT_s[:, h * T + qc * 512 : h * T + (qc + 1) * 512]
                    ps_st = stp.tile([128, 512], f32, tag="st", name=f"st_{ci}_{kt}")
                    nc.tensor.matmul(
                        ps_st[:],
                        kT_s[:, kt * 128 : (kt + 1) * 128],
                        qsl,
                        start=True, stop=True,
                    )
                    if use_mask:
                        mt = mpool.tile([128, 512], f32, tag="mt", name=f"mt_{ci}_{kt}")
                        nc.sync.dma_start(
                            mt[:],
                            maskT[kt * 128 : (kt + 1) * 128, qc * 512 : (qc + 1) * 512],
                        )
                        nc.vector.tensor_add(ps_st[:], ps_st[:], mt[:])
                    e = epool.tile([128, 512], mmdt, tag="e", name=f"e_{ci}_{kt}")
                    nc.scalar.activation(e[:], ps_st[:], Exp, scale=SCALE)
                    return e

                aN_store = {}

                def emit_normA(h0, h1, it):
                    # stage 1 of in-place attnT normalization: transpose two
                    # head tiles -> tokens on partitions, scale by recip
                    # (per-partition). Stage 2 (normB) runs in a later drain
                    # slot so PE never waits on the vector scale.
                    for hh in (h0, h1):
                        asl = attnT_s[:, hh * T + it * 128 : hh * T + (it + 1) * 128]
                        tr1 = ntrp.tile([128, 128], mmdt, tag="ntr", name=f"tr1_{hh}_{it}")
                        nc.tensor.transpose(tr1[:], asl, identm_s[:])
                        aN = anorm.tile([128, 128], mmdt, tag="an", name=f"aN_{hh}_{it}")
                        nc.vector.tensor_scalar_mul(
                            aN[:], tr1[:], rT_s[:, it * HPC + hh : it * HPC + hh + 1]
                        )
                        aN_store[(hh, it)] = aN

                def emit_normB(h0, h1, it):
                    # stage 2: transpose the scaled tiles back into attnT
                    for hh in (h0, h1):
                        aN = aN_store.pop((hh, it))
                        tr2 = ntrp.tile([128, 128], mmdt, tag="ntr", name=f"tr2_{hh}_{it}")
                        nc.tensor.transpose(tr2[:], aN[:], identm_s[:])
                        nc.vector.tensor_copy(
                            attnT_s[:, hh * T + it * 128 : hh * T + (it + 1) * 128],
                            tr2[:],
                        )

                def emit_p4_item(it, dc, drain_eng):
                    # 4 per-head Wo matmuls accumulate into one psum bank
                    p = wops.tile([128, 512], f32, tag="wo", name=f"wo_{it}_{dc}")
                    for h in range(HPC):
                        nc.tensor.matmul(
                            p[:],
                            attnT_s[:, h * T + it * 128 : h * T + (it + 1) * 128],
                            woT_s[:, h * D_MODEL + dc * 512 : h * D_MODEL + (dc + 1) * 512],
                            start=(h == 0), stop=(h == HPC - 1),
                        )
                    oo = omg.tile([128, 512], f32, tag="oo", name=f"oo_{it}_{dc}")
                    if drain_eng == "scalar":
                        nc.scalar.activation(oo[:], p[:], Copy)
                    else:
                        nc.vector.tensor_copy(oo[:], p[:])
                    nc.sync.dma_start(
                        out[it * 128 : (it + 1) * 128, dc * 512 : (dc + 1) * 512],
                        oo[:],
                    )

                def drain_unit(u):
                    kind = u[0]
                    if kind == "normA":
                        emit_normA(u[1], u[2], u[3])
                    elif kind == "normB":
                        emit_normB(u[1], u[2], u[3])
                    else:
                        emit_p4_item(u[1], u[2], u[3])

                p4q = []  # deferred norm/p4 work units
                es_cur = [emit_scores(0, kt) for kt in range(NKK)]
                for ci in range(len(chunks)):
                    qc, h = chunks[ci]
                    ps_pv = pvp.tile([128, 512], f32, tag="pv", name=f"pv_{ci}")
                    # sumexp: kt 0-5 summed on PE (ones-matmuls), kt 6-8 on
                    # gpsimd, kt 9-15 on vector; two folding matmuls at end.
                    ps_sum = sump.tile([1, 512], f32, tag="sum", name=f"sum_{ci}")
                    esumG = esump.tile([128, 512], f32, tag="esG", name=f"esG_{ci}")
                    esumV = esump.tile([128, 512], f32, tag="esV", name=f"esV_{ci}")
                    es_next = []
                    for kt in range(NKK):
                        st, sp = kt == 0, kt == NKK - 1
                        nc.tensor.matmul(
                            ps_pv[:],
                            v_s[:, kt * 128 : (kt + 1) * 128],
                            es_cur[kt][:],
                            start=st, stop=sp,
                        )
                        if kt >= 12:
                            # PE ones-matmuls last: the chunk end depends only
                            # on fresh PV output, never on lagging engines
                            nc.tensor.matmul(
                                ps_sum[:], onesb_s[:], es_cur[kt][:],
                                start=(kt == 12), stop=False,
                            )
                        elif kt == 1:
                            nc.gpsimd.tensor_add(esumG[:], es_cur[0][:], es_cur[1][:])
                        elif 2 <= kt <= 4:
                            nc.gpsimd.tensor_add(esumG[:], esumG[:], es_cur[kt][:])
                        elif kt == 6:
                            nc.vector.tensor_add(esumV[:], es_cur[5][:], es_cur[6][:])
                        elif kt >= 7:
                            nc.vector.tensor_add(esumV[:], esumV[:], es_cur[kt][:])
                        if sp:
                            # free the pv bank as soon as accumulation stops
                            nc.vector.tensor_copy(
                                attnT_s[:, h * T + qc * 512 : h * T + (qc + 1) * 512],
                                ps_pv[:],
                            )
                        if ci + 1 < len(chunks):
                            es_next.append(emit_scores(ci + 1, kt))
                        if p4q and kt % 2 == 1:
                            drain_unit(p4q.pop(0))
                    # fold the gpsimd/vector partial sums into ps_sum
                    nc.tensor.matmul(
                        ps_sum[:], onesf_s[:], esumG[:], start=False, stop=False,
                    )
                    nc.tensor.matmul(
                        ps_sum[:], onesf_s[:], esumV[:], start=False, stop=True,
                    )
                    nc.scalar.activation(
                        sumexp_s[32 * h : 32 * h + 1, qc * 512 : (qc + 1) * 512],
                        ps_sum[0:1, :],
                        Copy,
                    )
                    es_cur = es_next
                    if h == HPC - 1:
                        # phase 3 for this qc: transpose sumexp rows, recip.
                        # Heads live at partitions {0,32,64,96} (engine
                        # partition-start constraint), so transpose the full
                        # 128 partitions and recip the 4 head columns.
                        for tt in range(4):
                            it = qc * 4 + tt
                            ps_r = ntrp.tile([128, 128], f32, tag="ntr", name=f"psr_{it}")
                            nc.tensor.transpose(
                                ps_r[:],
                                sumexp_s[:, qc * 512 + tt * 128 : qc * 512 + (tt + 1) * 128],
                                identf_s[:],
                            )
                            for hh in range(HPC):
                                nc.vector.reciprocal(
                                    rT_s[:, it * HPC + hh : it * HPC + hh + 1],
                                    ps_r[:, 32 * hh : 32 * hh + 1],
                                )
                        for tt in range(4):
                            it = qc * 4 + tt
                            p4q.append(("normA", 0, 1, it))
                            p4q.append(("normA", 2, 3, it))
                            p4q.append(("normB", 0, 1, it))
                            p4q.append(("normB", 2, 3, it))
                            for dc in range(4):
                                p4q.append(("p4", it, dc, "vector"))
                for u in p4q:
                    drain_unit(u)

    _split_multi_waits(nc)
    return nc


_cache = {}


def _get_nc(mmdt_name, use_mask):
    key = (mmdt_name, use_mask)
    if key not in _cache:
        _cache[key] = _build(getattr(mybir.dt, mmdt_name), use_mask)
    return _cache[key]


def _np_dt(mmdt_name):
    if mmdt_name == "bfloat16":
        import ml_dtypes

        return ml_dtypes.bfloat16
    return np.float32


def _prep_inputs(x, attn_mask, Wq1, Wq2, Wk1, Wk2, Wv1, Wv2, Wo, mmdt_name):
    ndt = _np_dt(mmdt_name)
    identm = np.eye(128, dtype=np.float32).astype(ndt)
    identf = np.eye(128, dtype=np.float32)
    use_mask = bool(np.any(attn_mask))
    maskT = None
    if use_mask:
        maskT = np.ascontiguousarray(attn_mask[0, 0].T * np.sqrt(DH)).astype(
            np.float32
        )
    Wq1_64, Wq2_64 = Wq1.astype(np.float64), Wq2.astype(np.float64)
    Wk1_64, Wk2_64 = Wk1.astype(np.float64), Wk2.astype(np.float64)
    Wv1_64, Wv2_64 = Wv1.astype(np.float64), Wv2.astype(np.float64)
    xT_b = [
        np.ascontiguousarray(np.asarray(x[b]).T).astype(ndt) for b in range(B)
    ]
    in_maps = []
    for c in range(NCORES):
        b, g = divmod(c, NGROUP)
        h0 = g * HPC
        wq_f = (Wq2_64[h0 * DH : (h0 + HPC) * DH] @ Wq1_64).T  # [D, HPC*DH]
        wk_f = (Wk2_64[g * DH : (g + 1) * DH] @ Wk1_64).T      # [D, DH]
        wv_f = (Wv2_64[g * DH : (g + 1) * DH] @ Wv1_64).T
        woT_c = np.ascontiguousarray(Wo[:, h0 * DH : (h0 + HPC) * DH].T)
        m = {
            "xT": xT_b[b],
            "wq": np.ascontiguousarray(wq_f).astype(ndt),
            "wk": np.ascontiguousarray(wk_f).astype(ndt),
            "wv": np.ascontiguousarray(wv_f).astype(ndt),
            "woT": woT_c.astype(ndt),
            "identm": identm,
            "identf": identf,
        }
        if use_mask:
            m["maskT"] = maskT
        in_maps.append(m)
    return in_maps, use_mask


def run(x, attn_mask, Wq1, Wq2, Wk1, Wk2, Wv1, Wv2, Wo, **spmd_kwargs):
    mmdt_name = os.environ.get("BASS_MLA_DT", "bfloat16")
    in_maps, use_mask = _prep_inputs(
        x, attn_mask, Wq1, Wq2, Wk1, Wk2, Wv1, Wv2, Wo, mmdt_name
    )
    nc = _get_nc(mmdt_name, use_mask)
    res = bass_utils.run_bass_kernel_spmd(
        nc, in_maps, core_ids=list(range(NCORES)), **spmd_kwargs
    )
    out = np.zeros((B, T, D_MODEL), np.float64)
    for c in range(NCORES):
        out[c // NGROUP] += res.results[c]["out"]
    return out.astype(np.float32), res


def kernel(x, attn_mask, Wq1, Wq2, Wk1, Wk2, Wv1, Wv2, Wo):
    out, _ = run(x, attn_mask, Wq1, Wq2, Wk1, Wk2, Wv1, Wv2, Wo)
    return out


# revision 31
# speedup vs baseline: 1.0493x; 1.0260x over previous
"""MLA (low-rank QKV projection + GQA attention) Bass kernel for 8 trn2 cores.

Problem shapes (hardcoded):
  x [B=2, T=2048, D=2048], Wq1 [512,2048], Wq2 [2048,512],
  Wk1/Wv1 [256,2048], Wk2/Wv2 [512,256], Wo [2048,2048]
  HQ=16 q-heads, HKV=4 kv-heads (GROUP=4), DH=128.

Sharding: core c = (b, g) with b = c//4 (data-parallel over batch),
g = c%4 (tensor-parallel over head groups). Each core owns q-heads
{4g..4g+3} and kv-head g for its batch's 2048 tokens. Host folds
(W2_head @ W1) into per-head direct projections (exact math in float64),
pre-transposes x[b] to [D, T], and sums the 4 per-core partial Wo
outputs per batch.

Device per core:
  phase 1: qT [128, 4*2048], kT [128, 2048], vT [128, 2048] via folded
           weights; vT PE-transposed into v (key tokens on partitions),
           interleaved with the projection loop.
  phase 2: per (qc, h) chunk of 512 queries: scores^T = kT.T @ qT chunk,
           E = exp(scale*S) (ACT), PV accumulate + ones-row sumexp
           matmul, store unnormalized attnT (bf16), sumexp rows.
           Chunks are software-pipelined (next chunk's scores interleave
           with this chunk's PV) and ordered qc-major so phases 3/4 for
           qc can start while qc+1 is still in attention.
  phase 3: per qc: PE-transpose sumexp rows [4,128] -> [128,4], vector
           reciprocal -> rT (per-token-per-head normalizers).
  phase 4: per (token tile, dmodel chunk): 4 per-head Wo matmuls, then a
           fused scale-and-accumulate chain spread over scalar/vector/
           gpsimd engines, normalized output DMA'd to DRAM.
"""

import os
import numpy as np

import concourse.bass as bass
import concourse.tile as tile
from concourse import mybir
from concourse import bass_utils

D_MODEL, HQ, HKV, RQ, RKV = 2048, 16, 4, 512, 256
DH = D_MODEL // HQ            # 128
GROUP = HQ // HKV             # 4
B, T = 2, 2048
NCORES = 8
NGROUP = 4                    # tensor-parallel groups (one per kv head)
HPC = HQ // NGROUP            # 4 q-heads per core
SCALE = 1.0 / np.sqrt(DH)

NK = D_MODEL // 128           # 16 contraction tiles over D
NTC = T // 512                # 4 token chunks of 512
NKK = T // 128                # 16 key tiles of 128
NQC = T // 512                # 4 query chunks of 512
NTT = T // 128                # 16 token tiles of 128

f32 = mybir.dt.float32


class _TC(tile.TileContext):
    pass


_nop_ctr = [0]


def _split_multi_waits(nc):
    """This walrus build's CoreV3 lowering accepts only ONE sync-wait per
    instruction; move extra waits onto same-engine single-wait nops inserted
    immediately before the instruction."""
    for f in nc.m.functions:
        for bb in f.blocks:
            insts = list(bb.instructions)
            out = []
            changed = False
            for ins in insts:
                si = ins.sync_info
                if si is not None and si.on_wait and len(si.on_wait) > 1:
                    waits = list(si.on_wait)
                    for w in waits[:-1]:
                        _nop_ctr[0] += 1
                        nop = mybir.InstNoOp(
                            name=f"waitsplit_{_nop_ctr[0]}",
                            ins=[],
                            outs=[],
                            engine=ins.engine,
                        )
                        nop.sync_info = mybir.SyncInfo(on_wait=[w], on_update=[])
                        nc.register_instruction(nop)
                        out.append(nop)
                    ins.sync_info = mybir.SyncInfo(
                        on_wait=[waits[-1]], on_update=list(si.on_update)
                    )
                    changed = True
                out.append(ins)
            if changed:
                bb.instructions = out


def _build(mmdt, use_mask):
    nc = bass.Bass(trn_type="TRN2")
    xT = nc.dram_tensor("xT", (D_MODEL, T), mmdt, kind="ExternalInput")
    wq = nc.dram_tensor("wq", (D_MODEL, HPC * DH), mmdt, kind="ExternalInput")
    wk = nc.dram_tensor("wk", (D_MODEL, DH), mmdt, kind="ExternalInput")
    wv = nc.dram_tensor("wv", (D_MODEL, DH), mmdt, kind="ExternalInput")
    woT = nc.dram_tensor("woT", (HPC * DH, D_MODEL), mmdt, kind="ExternalInput")
    identm = nc.dram_tensor("identm", (128, 128), mmdt, kind="ExternalInput")
    identf = nc.dram_tensor("identf", (128, 128), f32, kind="ExternalInput")
    if use_mask:
        # pre-transposed, pre-scaled by sqrt(DH): [k, q]
        maskT = nc.dram_tensor("maskT", (T, T), f32, kind="ExternalInput")
    else:
        maskT = None
    out = nc.dram_tensor("out", (T, D_MODEL), f32, kind="ExternalOutput")

    Exp = mybir.ActivationFunctionType.Exp
    Copy = mybir.ActivationFunctionType.Copy
    Mult = mybir.AluOpType.mult
    Add = mybir.AluOpType.add

    with _TC(nc) as tc:
        with (
            tc.tile_pool(name="persist", bufs=1) as persist,
            tc.tile_pool(name="consts", bufs=1) as consts,
        ):
            qT_s = persist.tile([128, HPC * T], mmdt)     # head h at cols h*T
            kT_s = persist.tile([128, T], mmdt)
            vT_s = persist.tile([128, T], mmdt)
            v_s = persist.tile([128, T], mmdt)            # kk-tile t at cols t*128
            attnT_s = persist.tile([128, HPC * T], mmdt)  # unnormalized PV
            sumexp_s = persist.tile([128, T], f32)        # head h on partition 32*h
            rT_s = persist.tile([128, NTT * HPC], f32)    # recip, tok on partition
            woT_s = persist.tile([128, HPC * D_MODEL], mmdt)
            onesf_s = consts.tile([128, 1], f32)
            onesb_s = consts.tile([128, 1], mmdt)
            identm_s = consts.tile([128, 128], mmdt)
            identf_s = consts.tile([128, 128], f32)
            nc.vector.memset(onesf_s[:], 1.0)
            nc.vector.memset(onesb_s[:], 1.0)
            # unused partitions of sumexp_s flow through the phase-3
            # transpose; init so no garbage/non-finite values are read
            nc.vector.memset(sumexp_s[:], 1.0)

            # ---------------- phase 1: QKV projections ----------------
            # Weight/const DMA goes on the (otherwise idle) gpsimd queue;
            # sync queue carries x tiles. Per-kd slices so the first
            # matmuls can start early. Two passes (k/v then q) so every
            # psum tag is double-buffered within the 8-bank budget; x is
            # simply re-DMA'd for the second pass.
            with (
                tc.tile_pool(name="wgt", bufs=1) as wgt,
                tc.tile_pool(name="xin", bufs=4) as xin,
            ):
                wq_s = wgt.tile([128, NK * HPC * DH], mmdt)
                wk_s = wgt.tile([128, NK * DH], mmdt)
                wv_s = wgt.tile([128, NK * DH], mmdt)
                for g4 in range(NK // 4):
                    gsl = slice(g4 * 512, (g4 + 1) * 512)
                    nc.gpsimd.dma_start(
                        wk_s[:, g4 * 512 : (g4 + 1) * 512].rearrange(
                            "p (t m) -> p t m", t=4
                        ),
                        wk[gsl, :].rearrange("(t p) m -> p t m", p=128),
                    )
                    nc.gpsimd.dma_start(
                        wv_s[:, g4 * 512 : (g4 + 1) * 512].rearrange(
                            "p (t m) -> p t m", t=4
                        ),
                        wv[gsl, :].rearrange("(t p) m -> p t m", p=128),
                    )
                    if g4 == 0:
                        nc.gpsimd.dma_start(identm_s[:], identm[:])
                        nc.gpsimd.dma_start(identf_s[:], identf[:])
                for kd in range(NK):
                    nc.scalar.dma_start(
                        wq_s[:, kd * 512 : (kd + 1) * 512],
                        wq[kd * 128 : (kd + 1) * 128, :],
                    )
                for h in range(HPC):
                    nc.scalar.dma_start(
                        woT_s[:, h * D_MODEL : (h + 1) * D_MODEL],
                        woT[h * 128 : (h + 1) * 128, :],
                    )

                # pass A: k and v projections, v transposed as chunks finish
                with (
                    tc.tile_pool(name="kvp", bufs=2, space="PSUM") as kvp,
                    tc.tile_pool(name="trp", bufs=2, space="PSUM") as trp,
                ):
                    def emit_vtr(n):
                        for t in range(n * 4, n * 4 + 4):
                            tr = trp.tile([128, 128], mmdt, tag="tr", name=f"tr_{t}")
                            nc.tensor.transpose(
                                tr[:], vT_s[:, t * 128 : (t + 1) * 128], identm_s[:]
                            )
                            nc.vector.tensor_copy(
                                v_s[:, t * 128 : (t + 1) * 128], tr[:]
                            )

                    for n in range(NTC):
                        nsl = slice(n * 512, (n + 1) * 512)
                        ps_k = kvp.tile([128, 512], f32, tag="psk", name=f"psk_{n}")
                        ps_v = kvp.tile([128, 512], f32, tag="psv", name=f"psv_{n}")
                        for kd in range(NK):
                            xt = xin.tile([128, 512], mmdt, tag="xt", name=f"xta_{n}_{kd}")
                            nc.sync.dma_start(
                                xt[:], xT[kd * 128 : (kd + 1) * 128, nsl]
                            )
                            st, sp = kd == 0, kd == NK - 1
                            nc.tensor.matmul(
                                ps_k[:], wk_s[:, kd * 128 : (kd + 1) * 128], xt[:],
                                start=st, stop=sp,
                            )
                            nc.tensor.matmul(
                                ps_v[:], wv_s[:, kd * 128 : (kd + 1) * 128], xt[:],
                                start=st, stop=sp,
                            )
                        nc.scalar.activation(kT_s[:, nsl], ps_k[:], Copy)
                        nc.vector.tensor_copy(vT_s[:, nsl], ps_v[:])
                        if n > 0:
                            emit_vtr(n - 1)
                    emit_vtr(NTC - 1)

                # pass B: q projections, all four heads double-buffered
                with tc.tile_pool(name="qp", bufs=2, space="PSUM") as qp:
                    for n in range(NTC):
                        nsl = slice(n * 512, (n + 1) * 512)
                        ps_q = [
                            qp.tile([128, 512], f32, tag=f"psq{j}", name=f"psq{j}_{n}")
                            for j in range(HPC)
                        ]
                        for kd in range(NK):
                            xt = xin.tile([128, 512], mmdt, tag="xt", name=f"xtb_{n}_{kd}")
                            nc.sync.dma_start(
                                xt[:], xT[kd * 128 : (kd + 1) * 128, nsl]
                            )
                            st, sp = kd == 0, kd == NK - 1
                            for j in range(HPC):
                                nc.tensor.matmul(
                                    ps_q[j][:],
                                    wq_s[:, kd * 512 + j * 128 : kd * 512 + (j + 1) * 128],
                                    xt[:],
                                    start=st, stop=sp,
                                )
                        nc.scalar.activation(qT_s[:, 0 * T + n * 512 : 0 * T + (n + 1) * 512], ps_q[0][:], Copy)
                        nc.scalar.activation(qT_s[:, 1 * T + n * 512 : 1 * T + (n + 1) * 512], ps_q[1][:], Copy)
                        nc.vector.tensor_copy(qT_s[:, 2 * T + n * 512 : 2 * T + (n + 1) * 512], ps_q[2][:])
                        nc.vector.tensor_copy(qT_s[:, 3 * T + n * 512 : 3 * T + (n + 1) * 512], ps_q[3][:])

            # ---------------- phases 2+3+4 interleaved ----------------
            with (
                tc.tile_pool(name="epool", bufs=36) as epool,
                tc.tile_pool(name="esum", bufs=2) as esump,
                tc.tile_pool(name="omg", bufs=4) as omg,
                tc.tile_pool(name="anorm", bufs=6) as anorm,
                tc.tile_pool(name="mpool", bufs=3) as mpool,
                tc.tile_pool(name="stp", bufs=3, space="PSUM") as stp,
                tc.tile_pool(name="pvp", bufs=1, space="PSUM") as pvp,
                tc.tile_pool(name="sump", bufs=1, space="PSUM") as sump,
                tc.tile_pool(name="ntrp", bufs=2, space="PSUM") as ntrp,
                tc.tile_pool(name="wops", bufs=1, space="PSUM") as wops,
            ):
                chunks = [(qc, h) for qc in range(NQC) for h in range(HPC)]

                def emit_scores(ci, kt):
                    qc, h = chunks[ci]
                    qsl = qT_s[:, h * T + qc * 512 : h * T + (qc + 1) * 512]
                    ps_st = stp.tile([128, 512], f32, tag="st", name=f"st_{ci}_{kt}")
                    nc.tensor.matmul(
                        ps_st[:],
                        kT_s[:, kt * 128 : (kt + 1) * 128],
                        qsl,
                        start=True, stop=True,
                    )
                    if use_mask:
                        mt = mpool.tile([128, 512], f32, tag="mt", name=f"mt_{ci}_{kt}")
                        nc.sync.dma_start(
                            mt[:],
                            maskT[kt * 128 : (kt + 1) * 128, qc * 512 : (qc + 1) * 512],
                        )
                        nc.vector.tensor_add(ps_st[:], ps_st[:], mt[:])
                    e = epool.tile([128, 512], mmdt, tag="e", name=f"e_{ci}_{kt}")
                    nc.scalar.activation(e[:], ps_st[:], Exp, scale=SCALE)
                    return e

                aN_store = {}

                def emit_normA(h0, h1, it):
                    # stage 1 of in-place attnT normalization: transpose two
                    # head tiles -> tokens on partitions, scale by recip
                    # (per-partition). Stage 2 (normB) runs in a later drain
                    # slot so PE never waits on the vector scale.
                    for hh in (h0, h1):
                        asl = attnT_s[:, hh * T + it * 128 : hh * T + (it + 1) * 128]
                        tr1 = ntrp.tile([128, 128], mmdt, tag="ntr", name=f"tr1_{hh}_{it}")
                        nc.tensor.transpose(tr1[:], asl, identm_s[:])
                        aN = anorm.tile([128, 128], mmdt, tag="an", name=f"aN_{hh}_{it}")
                        nc.vector.tensor_scalar_mul(
                            aN[:], tr1[:], rT_s[:, it * HPC + hh : it * HPC + hh + 1]
                        )
                        aN_store[(hh, it)] = aN

                def emit_normB(h0, h1, it):
                    # stage 2: transpose the scaled tiles back into attnT
                    for hh in (h0, h1):
                        aN = aN_store.pop((hh, it))
                        tr2 = ntrp.tile([128, 128], mmdt, tag="ntr", name=f"tr2_{hh}_{it}")
                        nc.tensor.transpose(tr2[:], aN[:], identm_s[:])
                        nc.vector.tensor_copy(
                            attnT_s[:, hh * T + it * 128 : hh * T + (it + 1) * 128],
                            tr2[:],
                        )

                def emit_p4_item(it, dc, drain_eng):
                    # 4 per-head Wo matmuls accumulate into one psum bank
                    p = wops.tile([128, 512], f32, tag="wo", name=f"wo_{it}_{dc}")
                    for h in range(HPC):
                        nc.tensor.matmul(
                            p[:],
                            attnT_s[:, h * T + it * 128 : h * T + (it + 1) * 128],
                            woT_s[:, h * D_MODEL + dc * 512 : h * D_MODEL + (dc + 1) * 512],
                            start=(h == 0), stop=(h == HPC - 1),
                        )
                    oo = omg.tile([128, 512], f32, tag="oo", name=f"oo_{it}_{dc}")
                    if drain_eng == "scalar":
                        nc.scalar.activation(oo[:], p[:], Copy)
                    else:
                        nc.vector.tensor_copy(oo[:], p[:])
                    nc.sync.dma_start(
                        out[it * 128 : (it + 1) * 128, dc * 512 : (dc + 1) * 512],
                        oo[:],
                    )

                def drain_unit(u):
                    kind = u[0]
                    if kind == "normA":
                        emit_normA(u[1], u[2], u[3])
                    elif kind == "normB":
                        emit_normB(u[1], u[2], u[3])
                    else:
                        emit_p4_item(u[1], u[2], u[3])

                p4q = []  # deferred norm/p4 work units
                es_cur = [emit_scores(0, kt) for kt in range(NKK)]
                for ci in range(len(chunks)):
                    qc, h = chunks[ci]
                    ps_pv = pvp.tile([128, 512], f32, tag="pv", name=f"pv_{ci}")
                    # sumexp: kt 0-5 summed on PE (ones-matmuls), kt 6-8 on
                    # gpsimd, kt 9-15 on vector; two folding matmuls at end.
                    ps_sum = sump.tile([1, 512], f32, tag="sum", name=f"sum_{ci}")
                    esumG = esump.tile([128, 512], f32, tag="esG", name=f"esG_{ci}")
                    esumV = esump.tile([128, 512], f32, tag="esV", name=f"esV_{ci}")
                    es_next = []
                    for kt in range(NKK):
                        st, sp = kt == 0, kt == NKK - 1
                        nc.tensor.matmul(
                            ps_pv[:],
                            v_s[:, kt * 128 : (kt + 1) * 128],
                            es_cur[kt][:],
                            start=st, stop=sp,
                        )
                        if kt >= 12:
                            # PE ones-matmuls last: the chunk end depends only
                            # on fresh PV output, never on lagging engines
                            nc.tensor.matmul(
                                ps_sum[:], onesb_s[:], es_cur[kt][:],
                                start=(kt == 12), stop=False,
                            )
                        elif kt == 1:
                            nc.gpsimd.tensor_add(esumG[:], es_cur[0][:], es_cur[1][:])
                        elif 2 <= kt <= 4:
                            nc.gpsimd.tensor_add(esumG[:], esumG[:], es_cur[kt][:])
                        elif kt == 6:
                            nc.vector.tensor_add(esumV[:], es_cur[5][:], es_cur[6][:])
                        elif kt >= 7:
                            nc.vector.tensor_add(esumV[:], esumV[:], es_cur[kt][:])
                        if sp:
                            # free the pv bank as soon as accumulation stops
                            nc.vector.tensor_copy(
                                attnT_s[:, h * T + qc * 512 : h * T + (qc + 1) * 512],
                                ps_pv[:],
                            )
                        if ci + 1 < len(chunks):
                            es_next.append(emit_scores(ci + 1, kt))
                        if p4q and kt % 2 == 1:
                            drain_unit(p4q.pop(0))
                    # fold the gpsimd/vector partial sums into ps_sum
                    nc.tensor.matmul(
                        ps_sum[:], onesf_s[:], esumG[:], start=False, stop=False,
                    )
                    nc.tensor.matmul(
                        ps_sum[:], onesf_s[:], esumV[:], start=False, stop=True,
                    )
                    nc.scalar.activation(
                        sumexp_s[32 * h : 32 * h + 1, qc * 512 : (qc + 1) * 512],
                        ps_sum[0:1, :],
                        Copy,
                    )
                    es_cur = es_next
                    if h == HPC - 1:
                        # phase 3 for this qc: transpose sumexp rows, recip.
                        # Heads live at partitions {0,32,64,96} (engine
                        # partition-start constraint), so transpose the full
                        # 128 partitions and recip the 4 head columns.
                        for tt in range(4):
                            it = qc * 4 + tt
                            ps_r = ntrp.tile([128, 128], f32, tag="ntr", name=f"psr_{it}")
                            nc.tensor.transpose(
                                ps_r[:],
                                sumexp_s[:, qc * 512 + tt * 128 : qc * 512 + (tt + 1) * 128],
                                identf_s[:],
                            )
                            for hh in range(HPC):
                                nc.vector.reciprocal(
                                    rT_s[:, it * HPC + hh : it * HPC + hh + 1],
                                    ps_r[:, 32 * hh : 32 * hh + 1],
                                )
                        for tt in range(4):
                            it = qc * 4 + tt
                            p4q.append(("normA", 0, 1, it))
                            p4q.append(("normA", 2, 3, it))
                            p4q.append(("normB", 0, 1, it))
                            p4q.append(("normB", 2, 3, it))
                            for dc in range(4):
                                p4q.append(("p4", it, dc, "vector"))
                # tail: scalar is exp-free here, so alternate p4 drains
                n_p4 = 0
                for u in p4q:
                    if u[0] == "p4":
                        emit_p4_item(u[1], u[2], "scalar" if n_p4 % 2 else "vector")
                        n_p4 += 1
                    else:
                        drain_unit(u)

    _split_multi_waits(nc)
    return nc


_cache = {}


def _get_nc(mmdt_name, use_mask):
    key = (mmdt_name, use_mask)
    if key not in _cache:
        _cache[key] = _build(getattr(mybir.dt, mmdt_name), use_mask)
    return _cache[key]


def _np_dt(mmdt_name):
    if mmdt_name == "bfloat16":
        import ml_dtypes

        return ml_dtypes.bfloat16
    return np.float32


def _prep_inputs(x, attn_mask, Wq1, Wq2, Wk1, Wk2, Wv1, Wv2, Wo, mmdt_name):
    ndt = _np_dt(mmdt_name)
    identm = np.eye(128, dtype=np.float32).astype(ndt)
    identf = np.eye(128, dtype=np.float32)
    use_mask = bool(np.any(attn_mask))
    maskT = None
    if use_mask:
        maskT = np.ascontiguousarray(attn_mask[0, 0].T * np.sqrt(DH)).astype(
            np.float32
        )
    Wq1_64, Wq2_64 = Wq1.astype(np.float64), Wq2.astype(np.float64)
    Wk1_64, Wk2_64 = Wk1.astype(np.float64), Wk2.astype(np.float64)
    Wv1_64, Wv2_64 = Wv1.astype(np.float64), Wv2.astype(np.float64)
    xT_b = [
        np.ascontiguousarray(np.asarray(x[b]).T).astype(ndt) for b in range(B)
    ]
    in_maps = []
    for c in range(NCORES):
        b, g = divmod(c, NGROUP)
        h0 = g * HPC
        wq_f = (Wq2_64[h0 * DH : (h0 + HPC) * DH] @ Wq1_64).T  # [D, HPC*DH]
        wk_f = (Wk2_64[g * DH : (g + 1) * DH] @ Wk1_64).T      # [D, DH]
        wv_f = (Wv2_64[g * DH : (g + 1) * DH] @ Wv1_64).T
        woT_c = np.ascontiguousarray(Wo[:, h0 * DH : (h0 + HPC) * DH].T)
        m = {
            "xT": xT_b[b],
            "wq": np.ascontiguousarray(wq_f).astype(ndt),
            "wk": np.ascontiguousarray(wk_f).astype(ndt),
            "wv": np.ascontiguousarray(wv_f).astype(ndt),
            "woT": woT_c.astype(ndt),
            "identm": identm,
            "identf": identf,
        }
        if use_mask:
            m["maskT"] = maskT
        in_maps.append(m)
    return in_maps, use_mask


def run(x, attn_mask, Wq1, Wq2, Wk1, Wk2, Wv1, Wv2, Wo, **spmd_kwargs):
    mmdt_name = os.environ.get("BASS_MLA_DT", "bfloat16")
    in_maps, use_mask = _prep_inputs(
        x, attn_mask, Wq1, Wq2, Wk1, Wk2, Wv1, Wv2, Wo, mmdt_name
    )
    nc = _get_nc(mmdt_name, use_mask)
    res = bass_utils.run_bass_kernel_spmd(
        nc, in_maps, core_ids=list(range(NCORES)), **spmd_kwargs
    )
    out = np.zeros((B, T, D_MODEL), np.float64)
    for c in range(NCORES):
        out[c // NGROUP] += res.results[c]["out"]
    return out.astype(np.float32), res


def kernel(x, attn_mask, Wq1, Wq2, Wk1, Wk2, Wv1, Wv2, Wo):
    out, _ = run(x, attn_mask, Wq1, Wq2, Wk1, Wk2, Wv1, Wv2, Wo)
    return out


# revision 33
# speedup vs baseline: 1.2277x; 1.1700x over previous
"""MLA (low-rank QKV projection + GQA attention) Bass kernel for 8 trn2 cores.

Problem shapes (hardcoded):
  x [B=2, T=2048, D=2048], Wq1 [512,2048], Wq2 [2048,512],
  Wk1/Wv1 [256,2048], Wk2/Wv2 [512,256], Wo [2048,2048]
  HQ=16 q-heads, HKV=4 kv-heads (GROUP=4), DH=128.

Sharding: core c = (b, g) with b = c//4 (data-parallel over batch),
g = c%4 (tensor-parallel over head groups). Each core owns q-heads
{4g..4g+3} and kv-head g for its batch's 2048 tokens. Host folds
(W2_head @ W1) into per-head direct projections (exact math in float64),
pre-transposes x[b] to [D, T], and sums the 4 per-core partial Wo
outputs per batch.

Device per core:
  phase 1: qT [128, 4*2048], kT [128, 2048], vT [128, 2048] via folded
           weights; vT PE-transposed into v (key tokens on partitions),
           interleaved with the projection loop.
  phase 2: per (qc, h) chunk of 512 queries: scores^T = kT.T @ qT chunk,
           E = exp(scale*S) (ACT), PV accumulate + ones-row sumexp
           matmul, store unnormalized attnT (bf16), sumexp rows.
           Chunks are software-pipelined (next chunk's scores interleave
           with this chunk's PV) and ordered qc-major so phases 3/4 for
           qc can start while qc+1 is still in attention.
  phase 3: per qc: PE-transpose sumexp rows [4,128] -> [128,4], vector
           reciprocal -> rT (per-token-per-head normalizers).
  phase 4: per (token tile, dmodel chunk): 4 per-head Wo matmuls, then a
           fused scale-and-accumulate chain spread over scalar/vector/
           gpsimd engines, normalized output DMA'd to DRAM.
"""

import os
import numpy as np

import concourse.bass as bass
import concourse.tile as tile
from concourse import mybir
from concourse import bass_utils

D_MODEL, HQ, HKV, RQ, RKV = 2048, 16, 4, 512, 256
DH = D_MODEL // HQ            # 128
GROUP = HQ // HKV             # 4
B, T = 2, 2048
NCORES = 8
NGROUP = 4                    # tensor-parallel groups (one per kv head)
HPC = HQ // NGROUP            # 4 q-heads per core
SCALE = 1.0 / np.sqrt(DH)

NK = D_MODEL // 128           # 16 contraction tiles over D
NTC = T // 512                # 4 token chunks of 512
NKK = T // 128                # 16 key tiles of 128
NQC = T // 512                # 4 query chunks of 512
NTT = T // 128                # 16 token tiles of 128

f32 = mybir.dt.float32


class _TC(tile.TileContext):
    pass


_nop_ctr = [0]


def _split_multi_waits(nc):
    """This walrus build's CoreV3 lowering accepts only ONE sync-wait per
    instruction; move extra waits onto same-engine single-wait nops inserted
    immediately before the instruction."""
    for f in nc.m.functions:
        for bb in f.blocks:
            insts = list(bb.instructions)
            out = []
            changed = False
            for ins in insts:
                si = ins.sync_info
                if si is not None and si.on_wait and len(si.on_wait) > 1:
                    waits = list(si.on_wait)
                    for w in waits[:-1]:
                        _nop_ctr[0] += 1
                        nop = mybir.InstNoOp(
                            name=f"waitsplit_{_nop_ctr[0]}",
                            ins=[],
                            outs=[],
                            engine=ins.engine,
                        )
                        nop.sync_info = mybir.SyncInfo(on_wait=[w], on_update=[])
                        nc.register_instruction(nop)
                        out.append(nop)
                    ins.sync_info = mybir.SyncInfo(
                        on_wait=[waits[-1]], on_update=list(si.on_update)
                    )
                    changed = True
                out.append(ins)
            if changed:
                bb.instructions = out


def _build(mmdt, use_mask):
    nc = bass.Bass(trn_type="TRN2")
    xT = nc.dram_tensor("xT", (D_MODEL, T), mmdt, kind="ExternalInput")
    wq = nc.dram_tensor("wq", (D_MODEL, HPC * DH), mmdt, kind="ExternalInput")
    wk = nc.dram_tensor("wk", (D_MODEL, DH), mmdt, kind="ExternalInput")
    wv = nc.dram_tensor("wv", (D_MODEL, DH), mmdt, kind="ExternalInput")
    woT = nc.dram_tensor("woT", (HPC * DH, D_MODEL), mmdt, kind="ExternalInput")
    identm = nc.dram_tensor("identm", (128, 128), mmdt, kind="ExternalInput")
    identf = nc.dram_tensor("identf", (128, 128), f32, kind="ExternalInput")
    if use_mask:
        # pre-transposed, pre-scaled by sqrt(DH): [k, q]
        maskT = nc.dram_tensor("maskT", (T, T), f32, kind="ExternalInput")
    else:
        maskT = None
    out = nc.dram_tensor("out", (T, D_MODEL), f32, kind="ExternalOutput")

    Exp = mybir.ActivationFunctionType.Exp
    Copy = mybir.ActivationFunctionType.Copy
    Mult = mybir.AluOpType.mult
    Add = mybir.AluOpType.add

    with _TC(nc) as tc:
        with (
            tc.tile_pool(name="persist", bufs=1) as persist,
            tc.tile_pool(name="consts", bufs=1) as consts,
        ):
            qT_s = persist.tile([128, HPC * T], mmdt)     # head h at cols h*T
            kT_s = persist.tile([128, T], mmdt)
            vT_s = persist.tile([128, T], mmdt)
            v_s = persist.tile([128, T], mmdt)            # kk-tile t at cols t*128
            attnT_s = persist.tile([128, HPC * T], mmdt)  # unnormalized PV
            sumexp_s = persist.tile([128, T], f32)        # head h on partition 32*h
            rT_s = persist.tile([128, NTT * HPC], f32)    # recip, tok on partition
            woT_s = persist.tile([128, HPC * D_MODEL], mmdt)
            onesf_s = consts.tile([128, 1], f32)
            onesb_s = consts.tile([128, 1], mmdt)
            identm_s = consts.tile([128, 128], mmdt)
            identf_s = consts.tile([128, 128], f32)
            nc.vector.memset(onesf_s[:], 1.0)
            nc.vector.memset(onesb_s[:], 1.0)
            # unused partitions of sumexp_s flow through the phase-3
            # transpose; init so no garbage/non-finite values are read
            nc.vector.memset(sumexp_s[:], 1.0)

            # ---------------- phase 1: QKV projections ----------------
            # Weight/const DMA goes on the (otherwise idle) gpsimd queue;
            # sync queue carries x tiles. Per-kd slices so the first
            # matmuls can start early. Two passes (k/v then q) so every
            # psum tag is double-buffered within the 8-bank budget; x is
            # simply re-DMA'd for the second pass.
            with (
                tc.tile_pool(name="wgt", bufs=1) as wgt,
                tc.tile_pool(name="xin", bufs=4) as xin,
            ):
                wq_s = wgt.tile([128, NK * HPC * DH], mmdt)
                wk_s = wgt.tile([128, NK * DH], mmdt)
                wv_s = wgt.tile([128, NK * DH], mmdt)
                for g4 in range(NK // 4):
                    gsl = slice(g4 * 512, (g4 + 1) * 512)
                    nc.gpsimd.dma_start(
                        wk_s[:, g4 * 512 : (g4 + 1) * 512].rearrange(
                            "p (t m) -> p t m", t=4
                        ),
                        wk[gsl, :].rearrange("(t p) m -> p t m", p=128),
                    )
                    nc.gpsimd.dma_start(
                        wv_s[:, g4 * 512 : (g4 + 1) * 512].rearrange(
                            "p (t m) -> p t m", t=4
                        ),
                        wv[gsl, :].rearrange("(t p) m -> p t m", p=128),
                    )
                    if g4 == 0:
                        nc.gpsimd.dma_start(identm_s[:], identm[:])
                        nc.gpsimd.dma_start(identf_s[:], identf[:])
                for kd in range(NK):
                    nc.scalar.dma_start(
                        wq_s[:, kd * 512 : (kd + 1) * 512],
                        wq[kd * 128 : (kd + 1) * 128, :],
                    )
                for h in range(HPC):
                    nc.scalar.dma_start(
                        woT_s[:, h * D_MODEL : (h + 1) * D_MODEL],
                        woT[h * 128 : (h + 1) * 128, :],
                    )

                # single pass: all six projections accumulate concurrently
                # (6 psum banks), v transposed as each chunk's vT finishes
                with (
                    tc.tile_pool(name="qkvp", bufs=1, space="PSUM") as qkvp,
                    tc.tile_pool(name="trp", bufs=2, space="PSUM") as trp,
                ):
                    def emit_vtr(n):
                        for t in range(n * 4, n * 4 + 4):
                            tr = trp.tile([128, 128], mmdt, tag="tr", name=f"tr_{t}")
                            nc.tensor.transpose(
                                tr[:], vT_s[:, t * 128 : (t + 1) * 128], identm_s[:]
                            )
                            nc.vector.tensor_copy(
                                v_s[:, t * 128 : (t + 1) * 128], tr[:]
                            )

                    for n in range(NTC):
                        nsl = slice(n * 512, (n + 1) * 512)
                        ps_q = [
                            qkvp.tile([128, 512], f32, tag=f"psq{j}", name=f"psq{j}_{n}")
                            for j in range(HPC)
                        ]
                        ps_k = qkvp.tile([128, 512], f32, tag="psk", name=f"psk_{n}")
                        ps_v = qkvp.tile([128, 512], f32, tag="psv", name=f"psv_{n}")
                        for kd in range(NK):
                            xt = xin.tile([128, 512], mmdt, tag="xt", name=f"xt_{n}_{kd}")
                            nc.sync.dma_start(
                                xt[:], xT[kd * 128 : (kd + 1) * 128, nsl]
                            )
                            st, sp = kd == 0, kd == NK - 1
                            for j in range(HPC):
                                nc.tensor.matmul(
                                    ps_q[j][:],
                                    wq_s[:, kd * 512 + j * 128 : kd * 512 + (j + 1) * 128],
                                    xt[:],
                                    start=st, stop=sp,
                                )
                            nc.tensor.matmul(
                                ps_k[:], wk_s[:, kd * 128 : (kd + 1) * 128], xt[:],
                                start=st, stop=sp,
                            )
                            nc.tensor.matmul(
                                ps_v[:], wv_s[:, kd * 128 : (kd + 1) * 128], xt[:],
                                start=st, stop=sp,
                            )
                        nc.scalar.activation(qT_s[:, 0 * T + n * 512 : 0 * T + (n + 1) * 512], ps_q[0][:], Copy)
                        nc.scalar.activation(qT_s[:, 1 * T + n * 512 : 1 * T + (n + 1) * 512], ps_q[1][:], Copy)
                        nc.vector.tensor_copy(qT_s[:, 2 * T + n * 512 : 2 * T + (n + 1) * 512], ps_q[2][:])
                        nc.vector.tensor_copy(qT_s[:, 3 * T + n * 512 : 3 * T + (n + 1) * 512], ps_q[3][:])
                        nc.vector.tensor_copy(kT_s[:, nsl], ps_k[:])
                        nc.vector.tensor_copy(vT_s[:, nsl], ps_v[:])
                        if n > 0:
                            emit_vtr(n - 1)
                    emit_vtr(NTC - 1)

            # ---------------- phases 2+3+4 interleaved ----------------
            with (
                tc.tile_pool(name="epool", bufs=36) as epool,
                tc.tile_pool(name="esum", bufs=2) as esump,
                tc.tile_pool(name="omg", bufs=4) as omg,
                tc.tile_pool(name="anorm", bufs=6) as anorm,
                tc.tile_pool(name="mpool", bufs=3) as mpool,
                tc.tile_pool(name="stp", bufs=3, space="PSUM") as stp,
                tc.tile_pool(name="pvp", bufs=1, space="PSUM") as pvp,
                tc.tile_pool(name="sump", bufs=1, space="PSUM") as sump,
                tc.tile_pool(name="ntrp", bufs=2, space="PSUM") as ntrp,
                tc.tile_pool(name="wops", bufs=1, space="PSUM") as wops,
            ):
                chunks = [(qc, h) for qc in range(NQC) for h in range(HPC)]

                def emit_scores(ci, kt):
                    qc, h = chunks[ci]
                    qsl = qT_s[:, h * T + qc * 512 : h * T + (qc + 1) * 512]
                    ps_st = stp.tile([128, 512], f32, tag="st", name=f"st_{ci}_{kt}")
                    nc.tensor.matmul(
                        ps_st[:],
                        kT_s[:, kt * 128 : (kt + 1) * 128],
                        qsl,
                        start=True, stop=True,
                    )
                    if use_mask:
                        mt = mpool.tile([128, 512], f32, tag="mt", name=f"mt_{ci}_{kt}")
                        nc.sync.dma_start(
                            mt[:],
                            maskT[kt * 128 : (kt + 1) * 128, qc * 512 : (qc + 1) * 512],
                        )
                        nc.vector.tensor_add(ps_st[:], ps_st[:], mt[:])
                    e = epool.tile([128, 512], mmdt, tag="e", name=f"e_{ci}_{kt}")
                    nc.scalar.activation(e[:], ps_st[:], Exp, scale=SCALE)
                    return e

                aN_store = {}

                def emit_normA(h0, h1, it):
                    # stage 1 of in-place attnT normalization: transpose two
                    # head tiles -> tokens on partitions, scale by recip
                    # (per-partition). Stage 2 (normB) runs in a later drain
                    # slot so PE never waits on the vector scale.
                    for hh in (h0, h1):
                        asl = attnT_s[:, hh * T + it * 128 : hh * T + (it + 1) * 128]
                        tr1 = ntrp.tile([128, 128], mmdt, tag="ntr", name=f"tr1_{hh}_{it}")
                        nc.tensor.transpose(tr1[:], asl, identm_s[:])
                        aN = anorm.tile([128, 128], mmdt, tag="an", name=f"aN_{hh}_{it}")
                        nc.vector.tensor_scalar_mul(
                            aN[:], tr1[:], rT_s[:, it * HPC + hh : it * HPC + hh + 1]
                        )
                        aN_store[(hh, it)] = aN

                def emit_normB(h0, h1, it):
                    # stage 2: transpose the scaled tiles back into attnT
                    for hh in (h0, h1):
                        aN = aN_store.pop((hh, it))
                        tr2 = ntrp.tile([128, 128], mmdt, tag="ntr", name=f"tr2_{hh}_{it}")
                        nc.tensor.transpose(tr2[:], aN[:], identm_s[:])
                        nc.vector.tensor_copy(
                            attnT_s[:, hh * T + it * 128 : hh * T + (it + 1) * 128],
                            tr2[:],
                        )

                def emit_p4_item(it, dc, drain_eng, pool=None, tag="wo"):
                    # 4 per-head Wo matmuls accumulate into one psum bank
                    p = (pool or wops).tile(
                        [128, 512], f32, tag=tag, name=f"wo_{it}_{dc}"
                    )
                    for h in range(HPC):
                        nc.tensor.matmul(
                            p[:],
                            attnT_s[:, h * T + it * 128 : h * T + (it + 1) * 128],
                            woT_s[:, h * D_MODEL + dc * 512 : h * D_MODEL + (dc + 1) * 512],
                            start=(h == 0), stop=(h == HPC - 1),
                        )
                    oo = omg.tile([128, 512], f32, tag="oo", name=f"oo_{it}_{dc}")
                    if drain_eng == "scalar":
                        nc.scalar.activation(oo[:], p[:], Copy)
                    else:
                        nc.vector.tensor_copy(oo[:], p[:])
                    nc.sync.dma_start(
                        out[it * 128 : (it + 1) * 128, dc * 512 : (dc + 1) * 512],
                        oo[:],
                    )

                def drain_unit(u):
                    kind = u[0]
                    if kind == "normA":
                        emit_normA(u[1], u[2], u[3])
                    elif kind == "normB":
                        emit_normB(u[1], u[2], u[3])
                    else:
                        emit_p4_item(u[1], u[2], u[3])

                p4q = []  # deferred norm/p4 work units
                es_cur = [emit_scores(0, kt) for kt in range(NKK)]
                for ci in range(len(chunks)):
                    qc, h = chunks[ci]
                    ps_pv = pvp.tile([128, 512], f32, tag="pv", name=f"pv_{ci}")
                    # sumexp: kt 0-5 summed on PE (ones-matmuls), kt 6-8 on
                    # gpsimd, kt 9-15 on vector; two folding matmuls at end.
                    ps_sum = sump.tile([1, 512], f32, tag="sum", name=f"sum_{ci}")
                    esumG = esump.tile([128, 512], f32, tag="esG", name=f"esG_{ci}")
                    esumV = esump.tile([128, 512], f32, tag="esV", name=f"esV_{ci}")
                    es_next = []
                    for kt in range(NKK):
                        st, sp = kt == 0, kt == NKK - 1
                        nc.tensor.matmul(
                            ps_pv[:],
                            v_s[:, kt * 128 : (kt + 1) * 128],
                            es_cur[kt][:],
                            start=st, stop=sp,
                        )
                        if kt >= 12:
                            # PE ones-matmuls last: the chunk end depends only
                            # on fresh PV output, never on lagging engines
                            nc.tensor.matmul(
                                ps_sum[:], onesb_s[:], es_cur[kt][:],
                                start=(kt == 12), stop=False,
                            )
                        elif kt == 1:
                            nc.gpsimd.tensor_add(esumG[:], es_cur[0][:], es_cur[1][:])
                        elif 2 <= kt <= 4:
                            nc.gpsimd.tensor_add(esumG[:], esumG[:], es_cur[kt][:])
                        elif kt == 6:
                            nc.vector.tensor_add(esumV[:], es_cur[5][:], es_cur[6][:])
                        elif kt >= 7:
                            nc.vector.tensor_add(esumV[:], esumV[:], es_cur[kt][:])
                        if sp:
                            # free the pv bank as soon as accumulation stops
                            nc.vector.tensor_copy(
                                attnT_s[:, h * T + qc * 512 : h * T + (qc + 1) * 512],
                                ps_pv[:],
                            )
                        if ci + 1 < len(chunks):
                            es_next.append(emit_scores(ci + 1, kt))
                        if p4q and kt % 2 == 1:
                            drain_unit(p4q.pop(0))
                    # fold the gpsimd/vector partial sums into ps_sum
                    nc.tensor.matmul(
                        ps_sum[:], onesf_s[:], esumG[:], start=False, stop=False,
                    )
                    nc.tensor.matmul(
                        ps_sum[:], onesf_s[:], esumV[:], start=False, stop=True,
                    )
                    nc.scalar.activation(
                        sumexp_s[32 * h : 32 * h + 1, qc * 512 : (qc + 1) * 512],
                        ps_sum[0:1, :],
                        Copy,
                    )
                    es_cur = es_next
                    if h == HPC - 1:
                        # phase 3 for this qc: transpose sumexp rows, recip.
                        # Heads live at partitions {0,32,64,96} (engine
                        # partition-start constraint), so transpose the full
                        # 128 partitions and recip the 4 head columns.
                        for tt in range(4):
                            it = qc * 4 + tt
                            ps_r = ntrp.tile([128, 128], f32, tag="ntr", name=f"psr_{it}")
                            nc.tensor.transpose(
                                ps_r[:],
                                sumexp_s[:, qc * 512 + tt * 128 : qc * 512 + (tt + 1) * 128],
                                identf_s[:],
                            )
                            for hh in range(HPC):
                                nc.vector.reciprocal(
                                    rT_s[:, it * HPC + hh : it * HPC + hh + 1],
                                    ps_r[:, 32 * hh : 32 * hh + 1],
                                )
                        for tt in range(4):
                            it = qc * 4 + tt
                            p4q.append(("normA", 0, 1, it))
                            p4q.append(("normA", 2, 3, it))
                            p4q.append(("normB", 0, 1, it))
                            p4q.append(("normB", 2, 3, it))
                            for dc in range(4):
                                p4q.append(("p4", it, dc, "vector"))
                # tail: scalar is exp-free and the score/pv/sum banks are
                # done, so alternate drain engines and rotate p4 psums over
                # three banks to pipeline back-to-back items
                tail_psum = [(wops, "wo"), (sump, "sum"), (pvp, "pv")]
                n_p4 = 0
                for u in p4q:
                    if u[0] == "p4":
                        pool, tag = tail_psum[n_p4 % 3]
                        emit_p4_item(
                            u[1], u[2],
                            "scalar" if n_p4 % 2 else "vector",
                            pool=pool, tag=tag,
                        )
                        n_p4 += 1
                    else:
                        drain_unit(u)

    _split_multi_waits(nc)
    return nc


_cache = {}


def _get_nc(mmdt_name, use_mask):
    key = (mmdt_name, use_mask)
    if key not in _cache:
        _cache[key] = _build(getattr(mybir.dt, mmdt_name), use_mask)
    return _cache[key]


def _np_dt(mmdt_name):
    if mmdt_name == "bfloat16":
        import ml_dtypes

        return ml_dtypes.bfloat16
    return np.float32


def _prep_inputs(x, attn_mask, Wq1, Wq2, Wk1, Wk2, Wv1, Wv2, Wo, mmdt_name):
    ndt = _np_dt(mmdt_name)
    identm = np.eye(128, dtype=np.float32).astype(ndt)
    identf = np.eye(128, dtype=np.float32)
    use_mask = bool(np.any(attn_mask))
    maskT = None
    if use_mask:
        maskT = np.ascontiguousarray(attn_mask[0, 0].T * np.sqrt(DH)).astype(
            np.float32
        )
    Wq1_64, Wq2_64 = Wq1.astype(np.float64), Wq2.astype(np.float64)
    Wk1_64, Wk2_64 = Wk1.astype(np.float64), Wk2.astype(np.float64)
    Wv1_64, Wv2_64 = Wv1.astype(np.float64), Wv2.astype(np.float64)
    xT_b = [
        np.ascontiguousarray(np.asarray(x[b]).T).astype(ndt) for b in range(B)
    ]
    in_maps = []
    for c in range(NCORES):
        b, g = divmod(c, NGROUP)
        h0 = g * HPC
        wq_f = (Wq2_64[h0 * DH : (h0 + HPC) * DH] @ Wq1_64).T  # [D, HPC*DH]
        wk_f = (Wk2_64[g * DH : (g + 1) * DH] @ Wk1_64).T      # [D, DH]
        wv_f = (Wv2_64[g * DH : (g + 1) * DH] @ Wv1_64).T
        woT_c = np.ascontiguousarray(Wo[:, h0 * DH : (h0 + HPC) * DH].T)
        m = {
            "xT": xT_b[b],
            "wq": np.ascontiguousarray(wq_f).astype(ndt),
            "wk": np.ascontiguousarray(wk_f).astype(ndt),
            "wv": np.ascontiguousarray(wv_f).astype(ndt),
            "woT": woT_c.astype(ndt),
            "identm": identm,
            "identf": identf,
        }
        if use_mask:
            m["maskT"] = maskT
        in_maps.append(m)
    return in_maps, use_mask


def run(x, attn_mask, Wq1, Wq2, Wk1, Wk2, Wv1, Wv2, Wo, **spmd_kwargs):
    mmdt_name = os.environ.get("BASS_MLA_DT", "bfloat16")
    in_maps, use_mask = _prep_inputs(
        x, attn_mask, Wq1, Wq2, Wk1, Wk2, Wv1, Wv2, Wo, mmdt_name
    )
    nc = _get_nc(mmdt_name, use_mask)
    res = bass_utils.run_bass_kernel_spmd(
        nc, in_maps, core_ids=list(range(NCORES)), **spmd_kwargs
    )
    out = np.zeros((B, T, D_MODEL), np.float64)
    for c in range(NCORES):
        out[c // NGROUP] += res.results[c]["out"]
    return out.astype(np.float32), res


def kernel(x, attn_mask, Wq1, Wq2, Wk1, Wk2, Wv1, Wv2, Wo):
    out, _ = run(x, attn_mask, Wq1, Wq2, Wk1, Wk2, Wv1, Wv2, Wo)
    return out


# revision 34
# speedup vs baseline: 1.2390x; 1.0092x over previous
"""MLA (low-rank QKV projection + GQA attention) Bass kernel for 8 trn2 cores.

Problem shapes (hardcoded):
  x [B=2, T=2048, D=2048], Wq1 [512,2048], Wq2 [2048,512],
  Wk1/Wv1 [256,2048], Wk2/Wv2 [512,256], Wo [2048,2048]
  HQ=16 q-heads, HKV=4 kv-heads (GROUP=4), DH=128.

Sharding: core c = (b, g) with b = c//4 (data-parallel over batch),
g = c%4 (tensor-parallel over head groups). Each core owns q-heads
{4g..4g+3} and kv-head g for its batch's 2048 tokens. Host folds
(W2_head @ W1) into per-head direct projections (exact math in float64),
pre-transposes x[b] to [D, T], and sums the 4 per-core partial Wo
outputs per batch.

Device per core:
  phase 1: qT [128, 4*2048], kT [128, 2048], vT [128, 2048] via folded
           weights; vT PE-transposed into v (key tokens on partitions),
           interleaved with the projection loop.
  phase 2: per (qc, h) chunk of 512 queries: scores^T = kT.T @ qT chunk,
           E = exp(scale*S) (ACT), PV accumulate + ones-row sumexp
           matmul, store unnormalized attnT (bf16), sumexp rows.
           Chunks are software-pipelined (next chunk's scores interleave
           with this chunk's PV) and ordered qc-major so phases 3/4 for
           qc can start while qc+1 is still in attention.
  phase 3: per qc: PE-transpose sumexp rows [4,128] -> [128,4], vector
           reciprocal -> rT (per-token-per-head normalizers).
  phase 4: per (token tile, dmodel chunk): 4 per-head Wo matmuls, then a
           fused scale-and-accumulate chain spread over scalar/vector/
           gpsimd engines, normalized output DMA'd to DRAM.
"""

import os
import numpy as np

import concourse.bass as bass
import concourse.tile as tile
from concourse import mybir
from concourse import bass_utils

D_MODEL, HQ, HKV, RQ, RKV = 2048, 16, 4, 512, 256
DH = D_MODEL // HQ            # 128
GROUP = HQ // HKV             # 4
B, T = 2, 2048
NCORES = 8
NGROUP = 4                    # tensor-parallel groups (one per kv head)
HPC = HQ // NGROUP            # 4 q-heads per core
SCALE = 1.0 / np.sqrt(DH)

NK = D_MODEL // 128           # 16 contraction tiles over D
NTC = T // 512                # 4 token chunks of 512
NKK = T // 128                # 16 key tiles of 128
NQC = T // 512                # 4 query chunks of 512
NTT = T // 128                # 16 token tiles of 128

f32 = mybir.dt.float32


class _TC(tile.TileContext):
    pass


_nop_ctr = [0]


def _split_multi_waits(nc):
    """This walrus build's CoreV3 lowering accepts only ONE sync-wait per
    instruction; move extra waits onto same-engine single-wait nops inserted
    immediately before the instruction."""
    for f in nc.m.functions:
        for bb in f.blocks:
            insts = list(bb.instructions)
            out = []
            changed = False
            for ins in insts:
                si = ins.sync_info
                if si is not None and si.on_wait and len(si.on_wait) > 1:
                    waits = list(si.on_wait)
                    for w in waits[:-1]:
                        _nop_ctr[0] += 1
                        nop = mybir.InstNoOp(
                            name=f"waitsplit_{_nop_ctr[0]}",
                            ins=[],
                            outs=[],
                            engine=ins.engine,
                        )
                        nop.sync_info = mybir.SyncInfo(on_wait=[w], on_update=[])
                        nc.register_instruction(nop)
                        out.append(nop)
                    ins.sync_info = mybir.SyncInfo(
                        on_wait=[waits[-1]], on_update=list(si.on_update)
                    )
                    changed = True
                out.append(ins)
            if changed:
                bb.instructions = out


def _build(mmdt, use_mask):
    nc = bass.Bass(trn_type="TRN2")
    xT = nc.dram_tensor("xT", (D_MODEL, T), mmdt, kind="ExternalInput")
    wq = nc.dram_tensor("wq", (D_MODEL, HPC * DH), mmdt, kind="ExternalInput")
    wk = nc.dram_tensor("wk", (D_MODEL, DH), mmdt, kind="ExternalInput")
    wv = nc.dram_tensor("wv", (D_MODEL, DH), mmdt, kind="ExternalInput")
    woT = nc.dram_tensor("woT", (HPC * DH, D_MODEL), mmdt, kind="ExternalInput")
    identm = nc.dram_tensor("identm", (128, 128), mmdt, kind="ExternalInput")
    identf = nc.dram_tensor("identf", (128, 128), f32, kind="ExternalInput")
    if use_mask:
        # pre-transposed, pre-scaled by sqrt(DH): [k, q]
        maskT = nc.dram_tensor("maskT", (T, T), f32, kind="ExternalInput")
    else:
        maskT = None
    out = nc.dram_tensor("out", (T, D_MODEL), f32, kind="ExternalOutput")

    Exp = mybir.ActivationFunctionType.Exp
    Copy = mybir.ActivationFunctionType.Copy
    Mult = mybir.AluOpType.mult
    Add = mybir.AluOpType.add

    with _TC(nc) as tc:
        with (
            tc.tile_pool(name="persist", bufs=1) as persist,
            tc.tile_pool(name="consts", bufs=1) as consts,
        ):
            qT_s = persist.tile([128, HPC * T], mmdt)     # head h at cols h*T
            kT_s = persist.tile([128, T], mmdt)
            vT_s = persist.tile([128, T], mmdt)
            v_s = persist.tile([128, T], mmdt)            # kk-tile t at cols t*128
            attnT_s = persist.tile([128, HPC * T], mmdt)  # unnormalized PV
            sumexp_s = persist.tile([128, T], f32)        # head h on partition 32*h
            rT_s = persist.tile([128, NTT * HPC], f32)    # recip, tok on partition
            woT_s = persist.tile([128, HPC * D_MODEL], mmdt)
            onesf_s = consts.tile([128, 1], f32)
            onesb_s = consts.tile([128, 1], mmdt)
            identm_s = consts.tile([128, 128], mmdt)
            identf_s = consts.tile([128, 128], f32)
            nc.vector.memset(onesf_s[:], 1.0)
            nc.vector.memset(onesb_s[:], 1.0)
            # unused partitions of sumexp_s flow through the phase-3
            # transpose; init so no garbage/non-finite values are read
            nc.vector.memset(sumexp_s[:], 1.0)

            # ---------------- phase 1: QKV projections ----------------
            # Weight/const DMA goes on the (otherwise idle) gpsimd queue;
            # sync queue carries x tiles. Per-kd slices so the first
            # matmuls can start early. Two passes (k/v then q) so every
            # psum tag is double-buffered within the 8-bank budget; x is
            # simply re-DMA'd for the second pass.
            with (
                tc.tile_pool(name="wgt", bufs=1) as wgt,
                tc.tile_pool(name="xin", bufs=4) as xin,
            ):
                wq_s = wgt.tile([128, NK * HPC * DH], mmdt)
                wk_s = wgt.tile([128, NK * DH], mmdt)
                wv_s = wgt.tile([128, NK * DH], mmdt)
                for kd in range(4):
                    ksl = slice(kd * 128, (kd + 1) * 128)
                    nc.gpsimd.dma_start(
                        wk_s[:, kd * 128 : (kd + 1) * 128], wk[ksl, :]
                    )
                    nc.gpsimd.dma_start(
                        wv_s[:, kd * 128 : (kd + 1) * 128], wv[ksl, :]
                    )
                nc.gpsimd.dma_start(identm_s[:], identm[:])
                nc.gpsimd.dma_start(identf_s[:], identf[:])
                for g4 in range(1, NK // 4):
                    gsl = slice(g4 * 512, (g4 + 1) * 512)
                    nc.gpsimd.dma_start(
                        wk_s[:, g4 * 512 : (g4 + 1) * 512].rearrange(
                            "p (t m) -> p t m", t=4
                        ),
                        wk[gsl, :].rearrange("(t p) m -> p t m", p=128),
                    )
                    nc.gpsimd.dma_start(
                        wv_s[:, g4 * 512 : (g4 + 1) * 512].rearrange(
                            "p (t m) -> p t m", t=4
                        ),
                        wv[gsl, :].rearrange("(t p) m -> p t m", p=128),
                    )
                for kd in range(NK):
                    nc.scalar.dma_start(
                        wq_s[:, kd * 512 : (kd + 1) * 512],
                        wq[kd * 128 : (kd + 1) * 128, :],
                    )
                for h in range(HPC):
                    nc.scalar.dma_start(
                        woT_s[:, h * D_MODEL : (h + 1) * D_MODEL],
                        woT[h * 128 : (h + 1) * 128, :],
                    )

                # single pass: all six projections accumulate concurrently
                # (6 psum banks), v transposed as each chunk's vT finishes
                with (
                    tc.tile_pool(name="qkvp", bufs=1, space="PSUM") as qkvp,
                    tc.tile_pool(name="trp", bufs=2, space="PSUM") as trp,
                ):
                    def emit_vtr(n):
                        for t in range(n * 4, n * 4 + 4):
                            tr = trp.tile([128, 128], mmdt, tag="tr", name=f"tr_{t}")
                            nc.tensor.transpose(
                                tr[:], vT_s[:, t * 128 : (t + 1) * 128], identm_s[:]
                            )
                            nc.vector.tensor_copy(
                                v_s[:, t * 128 : (t + 1) * 128], tr[:]
                            )

                    for n in range(NTC):
                        nsl = slice(n * 512, (n + 1) * 512)
                        ps_q = [
                            qkvp.tile([128, 512], f32, tag=f"psq{j}", name=f"psq{j}_{n}")
                            for j in range(HPC)
                        ]
                        ps_k = qkvp.tile([128, 512], f32, tag="psk", name=f"psk_{n}")
                        ps_v = qkvp.tile([128, 512], f32, tag="psv", name=f"psv_{n}")
                        for kd in range(NK):
                            xt = xin.tile([128, 512], mmdt, tag="xt", name=f"xt_{n}_{kd}")
                            nc.sync.dma_start(
                                xt[:], xT[kd * 128 : (kd + 1) * 128, nsl]
                            )
                            st, sp = kd == 0, kd == NK - 1
                            for j in range(HPC):
                                nc.tensor.matmul(
                                    ps_q[j][:],
                                    wq_s[:, kd * 512 + j * 128 : kd * 512 + (j + 1) * 128],
                                    xt[:],
                                    start=st, stop=sp,
                                )
                            nc.tensor.matmul(
                                ps_k[:], wk_s[:, kd * 128 : (kd + 1) * 128], xt[:],
                                start=st, stop=sp,
                            )
                            nc.tensor.matmul(
                                ps_v[:], wv_s[:, kd * 128 : (kd + 1) * 128], xt[:],
                                start=st, stop=sp,
                            )
                        nc.scalar.activation(qT_s[:, 0 * T + n * 512 : 0 * T + (n + 1) * 512], ps_q[0][:], Copy)
                        nc.scalar.activation(qT_s[:, 1 * T + n * 512 : 1 * T + (n + 1) * 512], ps_q[1][:], Copy)
                        nc.vector.tensor_copy(qT_s[:, 2 * T + n * 512 : 2 * T + (n + 1) * 512], ps_q[2][:])
                        nc.vector.tensor_copy(qT_s[:, 3 * T + n * 512 : 3 * T + (n + 1) * 512], ps_q[3][:])
                        nc.vector.tensor_copy(kT_s[:, nsl], ps_k[:])
                        nc.vector.tensor_copy(vT_s[:, nsl], ps_v[:])
                        if n > 0:
                            emit_vtr(n - 1)
                    emit_vtr(NTC - 1)

            # ---------------- phases 2+3+4 interleaved ----------------
            with (
                tc.tile_pool(name="epool", bufs=36) as epool,
                tc.tile_pool(name="esum", bufs=2) as esump,
                tc.tile_pool(name="omg", bufs=4) as omg,
                tc.tile_pool(name="anorm", bufs=6) as anorm,
                tc.tile_pool(name="mpool", bufs=3) as mpool,
                tc.tile_pool(name="stp", bufs=3, space="PSUM") as stp,
                tc.tile_pool(name="pvp", bufs=1, space="PSUM") as pvp,
                tc.tile_pool(name="sump", bufs=1, space="PSUM") as sump,
                tc.tile_pool(name="ntrp", bufs=2, space="PSUM") as ntrp,
                tc.tile_pool(name="wops", bufs=1, space="PSUM") as wops,
            ):
                chunks = [(qc, h) for qc in range(NQC) for h in range(HPC)]

                def emit_scores(ci, kt):
                    qc, h = chunks[ci]
                    qsl = qT_s[:, h * T + qc * 512 : h * T + (qc + 1) * 512]
                    ps_st = stp.tile([128, 512], f32, tag="st", name=f"st_{ci}_{kt}")
                    nc.tensor.matmul(
                        ps_st[:],
                        kT_s[:, kt * 128 : (kt + 1) * 128],
                        qsl,
                        start=True, stop=True,
                    )
                    if use_mask:
                        mt = mpool.tile([128, 512], f32, tag="mt", name=f"mt_{ci}_{kt}")
                        nc.sync.dma_start(
                            mt[:],
                            maskT[kt * 128 : (kt + 1) * 128, qc * 512 : (qc + 1) * 512],
                        )
                        nc.vector.tensor_add(ps_st[:], ps_st[:], mt[:])
                    e = epool.tile([128, 512], mmdt, tag="e", name=f"e_{ci}_{kt}")
                    nc.scalar.activation(e[:], ps_st[:], Exp, scale=SCALE)
                    return e

                aN_store = {}

                def emit_normA(h0, h1, it):
                    # stage 1 of in-place attnT normalization: transpose two
                    # head tiles -> tokens on partitions, scale by recip
                    # (per-partition). Stage 2 (normB) runs in a later drain
                    # slot so PE never waits on the vector scale.
                    for hh in (h0, h1):
                        asl = attnT_s[:, hh * T + it * 128 : hh * T + (it + 1) * 128]
                        tr1 = ntrp.tile([128, 128], mmdt, tag="ntr", name=f"tr1_{hh}_{it}")
                        nc.tensor.transpose(tr1[:], asl, identm_s[:])
                        aN = anorm.tile([128, 128], mmdt, tag="an", name=f"aN_{hh}_{it}")
                        nc.vector.tensor_scalar_mul(
                            aN[:], tr1[:], rT_s[:, it * HPC + hh : it * HPC + hh + 1]
                        )
                        aN_store[(hh, it)] = aN

                def emit_normB(h0, h1, it):
                    # stage 2: transpose the scaled tiles back into attnT
                    for hh in (h0, h1):
                        aN = aN_store.pop((hh, it))
                        tr2 = ntrp.tile([128, 128], mmdt, tag="ntr", name=f"tr2_{hh}_{it}")
                        nc.tensor.transpose(tr2[:], aN[:], identm_s[:])
                        nc.vector.tensor_copy(
                            attnT_s[:, hh * T + it * 128 : hh * T + (it + 1) * 128],
                            tr2[:],
                        )

                def emit_p4_item(it, dc, drain_eng, pool=None, tag="wo"):
                    # 4 per-head Wo matmuls accumulate into one psum bank
                    p = (pool or wops).tile(
                        [128, 512], f32, tag=tag, name=f"wo_{it}_{dc}"
                    )
                    for h in range(HPC):
                        nc.tensor.matmul(
                            p[:],
                            attnT_s[:, h * T + it * 128 : h * T + (it + 1) * 128],
                            woT_s[:, h * D_MODEL + dc * 512 : h * D_MODEL + (dc + 1) * 512],
                            start=(h == 0), stop=(h == HPC - 1),
                        )
                    oo = omg.tile([128, 512], f32, tag="oo", name=f"oo_{it}_{dc}")
                    if drain_eng == "scalar":
                        nc.scalar.activation(oo[:], p[:], Copy)
                    else:
                        nc.vector.tensor_copy(oo[:], p[:])
                    nc.sync.dma_start(
                        out[it * 128 : (it + 1) * 128, dc * 512 : (dc + 1) * 512],
                        oo[:],
                    )

                def drain_unit(u):
                    kind = u[0]
                    if kind == "normA":
                        emit_normA(u[1], u[2], u[3])
                    elif kind == "normB":
                        emit_normB(u[1], u[2], u[3])
                    else:
                        emit_p4_item(u[1], u[2], u[3])

                p4q = []  # deferred norm/p4 work units
                es_cur = [emit_scores(0, kt) for kt in range(NKK)]
                for ci in range(len(chunks)):
                    qc, h = chunks[ci]
                    ps_pv = pvp.tile([128, 512], f32, tag="pv", name=f"pv_{ci}")
                    # sumexp: kt 0-5 summed on PE (ones-matmuls), kt 6-8 on
                    # gpsimd, kt 9-15 on vector; two folding matmuls at end.
                    ps_sum = sump.tile([1, 512], f32, tag="sum", name=f"sum_{ci}")
                    esumG = esump.tile([128, 512], f32, tag="esG", name=f"esG_{ci}")
                    esumV = esump.tile([128, 512], f32, tag="esV", name=f"esV_{ci}")
                    es_next = []
                    for kt in range(NKK):
                        st, sp = kt == 0, kt == NKK - 1
                        nc.tensor.matmul(
                            ps_pv[:],
                            v_s[:, kt * 128 : (kt + 1) * 128],
                            es_cur[kt][:],
                            start=st, stop=sp,
                        )
                        if kt >= 12:
                            # PE ones-matmuls last: the chunk end depends only
                            # on fresh PV output, never on lagging engines
                            nc.tensor.matmul(
                                ps_sum[:], onesb_s[:], es_cur[kt][:],
                                start=(kt == 12), stop=False,
                            )
                        elif kt == 1:
                            nc.gpsimd.tensor_add(esumG[:], es_cur[0][:], es_cur[1][:])
                        elif 2 <= kt <= 4:
                            nc.gpsimd.tensor_add(esumG[:], esumG[:], es_cur[kt][:])
                        elif kt == 6:
                            nc.vector.tensor_add(esumV[:], es_cur[5][:], es_cur[6][:])
                        elif kt >= 7:
                            nc.vector.tensor_add(esumV[:], esumV[:], es_cur[kt][:])
                        if sp:
                            # free the pv bank as soon as accumulation stops
                            nc.vector.tensor_copy(
                                attnT_s[:, h * T + qc * 512 : h * T + (qc + 1) * 512],
                                ps_pv[:],
                            )
                        if ci + 1 < len(chunks):
                            es_next.append(emit_scores(ci + 1, kt))
                        if p4q and kt % 2 == 1:
                            drain_unit(p4q.pop(0))
                    # fold the gpsimd/vector partial sums into ps_sum
                    nc.tensor.matmul(
                        ps_sum[:], onesf_s[:], esumG[:], start=False, stop=False,
                    )
                    nc.tensor.matmul(
                        ps_sum[:], onesf_s[:], esumV[:], start=False, stop=True,
                    )
                    nc.scalar.activation(
                        sumexp_s[32 * h : 32 * h + 1, qc * 512 : (qc + 1) * 512],
                        ps_sum[0:1, :],
                        Copy,
                    )
                    es_cur = es_next
                    if h == HPC - 1:
                        # phase 3 for this qc: transpose sumexp rows, recip.
                        # Heads live at partitions {0,32,64,96} (engine
                        # partition-start constraint), so transpose the full
                        # 128 partitions and recip the 4 head columns.
                        for tt in range(4):
                            it = qc * 4 + tt
                            ps_r = ntrp.tile([128, 128], f32, tag="ntr", name=f"psr_{it}")
                            nc.tensor.transpose(
                                ps_r[:],
                                sumexp_s[:, qc * 512 + tt * 128 : qc * 512 + (tt + 1) * 128],
                                identf_s[:],
                            )
                            for hh in range(HPC):
                                nc.vector.reciprocal(
                                    rT_s[:, it * HPC + hh : it * HPC + hh + 1],
                                    ps_r[:, 32 * hh : 32 * hh + 1],
                                )
                        if qc < NQC - 1:
                            for tt in range(4):
                                it = qc * 4 + tt
                                p4q.append(("normA", 0, 1, it))
                                p4q.append(("normA", 2, 3, it))
                                p4q.append(("normB", 0, 1, it))
                                p4q.append(("normB", 2, 3, it))
                                for dc in range(4):
                                    p4q.append(("p4", it, dc, "vector"))
                        else:
                            # tail qc: pipeline across token tiles so p4
                            # items always have independent norm work of the
                            # next tile between dependent stages
                            base = qc * 4
                            p4q.append(("normA", 0, 1, base))
                            p4q.append(("normA", 2, 3, base))
                            p4q.append(("normB", 0, 1, base))
                            p4q.append(("normB", 2, 3, base))
                            for tt in range(1, 5):
                                if tt < 4:
                                    it = base + tt
                                    p4q.append(("normA", 0, 1, it))
                                    p4q.append(("normA", 2, 3, it))
                                for dc in range(4):
                                    p4q.append(("p4", base + tt - 1, dc, "vector"))
                                if tt < 4:
                                    it = base + tt
                                    p4q.append(("normB", 0, 1, it))
                                    p4q.append(("normB", 2, 3, it))
                # tail: scalar is exp-free and the score/pv/sum banks are
                # done, so alternate drain engines and rotate p4 psums over
                # three banks to pipeline back-to-back items
                tail_psum = [(wops, "wo"), (sump, "sum"), (pvp, "pv")]
                n_p4 = 0
                for u in p4q:
                    if u[0] == "p4":
                        pool, tag = tail_psum[n_p4 % 3]
                        emit_p4_item(
                            u[1], u[2],
                            "scalar" if n_p4 % 2 else "vector",
                            pool=pool, tag=tag,
                        )
                        n_p4 += 1
                    else:
                        drain_unit(u)

    _split_multi_waits(nc)
    return nc


_cache = {}


def _get_nc(mmdt_name, use_mask):
    key = (mmdt_name, use_mask)
    if key not in _cache:
        _cache[key] = _build(getattr(mybir.dt, mmdt_name), use_mask)
    return _cache[key]


def _np_dt(mmdt_name):
    if mmdt_name == "bfloat16":
        import ml_dtypes

        return ml_dtypes.bfloat16
    return np.float32


def _prep_inputs(x, attn_mask, Wq1, Wq2, Wk1, Wk2, Wv1, Wv2, Wo, mmdt_name):
    ndt = _np_dt(mmdt_name)
    identm = np.eye(128, dtype=np.float32).astype(ndt)
    identf = np.eye(128, dtype=np.float32)
    use_mask = bool(np.any(attn_mask))
    maskT = None
    if use_mask:
        maskT = np.ascontiguousarray(attn_mask[0, 0].T * np.sqrt(DH)).astype(
            np.float32
        )
    Wq1_64, Wq2_64 = Wq1.astype(np.float64), Wq2.astype(np.float64)
    Wk1_64, Wk2_64 = Wk1.astype(np.float64), Wk2.astype(np.float64)
    Wv1_64, Wv2_64 = Wv1.astype(np.float64), Wv2.astype(np.float64)
    xT_b = [
        np.ascontiguousarray(np.asarray(x[b]).T).astype(ndt) for b in range(B)
    ]
    in_maps = []
    for c in range(NCORES):
        b, g = divmod(c, NGROUP)
        h0 = g * HPC
        wq_f = (Wq2_64[h0 * DH : (h0 + HPC) * DH] @ Wq1_64).T  # [D, HPC*DH]
        wk_f = (Wk2_64[g * DH : (g + 1) * DH] @ Wk1_64).T      # [D, DH]
        wv_f = (Wv2_64[g * DH : (g + 1) * DH] @ Wv1_64).T
        woT_c = np.ascontiguousarray(Wo[:, h0 * DH : (h0 + HPC) * DH].T)
        m = {
            "xT": xT_b[b],
            "wq": np.ascontiguousarray(wq_f).astype(ndt),
            "wk": np.ascontiguousarray(wk_f).astype(ndt),
            "wv": np.ascontiguousarray(wv_f).astype(ndt),
            "woT": woT_c.astype(ndt),
            "identm": identm,
            "identf": identf,
        }
        if use_mask:
            m["maskT"] = maskT
        in_maps.append(m)
    return in_maps, use_mask


def run(x, attn_mask, Wq1, Wq2, Wk1, Wk2, Wv1, Wv2, Wo, **spmd_kwargs):
    mmdt_name = os.environ.get("BASS_MLA_DT", "bfloat16")
    in_maps, use_mask = _prep_inputs(
        x, attn_mask, Wq1, Wq2, Wk1, Wk2, Wv1, Wv2, Wo, mmdt_name
    )
    nc = _get_nc(mmdt_name, use_mask)
    res = bass_utils.run_bass_kernel_spmd(
        nc, in_maps, core_ids=list(range(NCORES)), **spmd_kwargs
    )
    out = np.zeros((B, T, D_MODEL), np.float64)
    for c in range(NCORES):
        out[c // NGROUP] += res.results[c]["out"]
    return out.astype(np.float32), res


def kernel(x, attn_mask, Wq1, Wq2, Wk1, Wk2, Wv1, Wv2, Wo):
    out, _ = run(x, attn_mask, Wq1, Wq2, Wk1, Wk2, Wv1, Wv2, Wo)
    return out


# revision 35
# speedup vs baseline: 1.2561x; 1.0138x over previous
"""MLA (low-rank QKV projection + GQA attention) Bass kernel for 8 trn2 cores.

Problem shapes (hardcoded):
  x [B=2, T=2048, D=2048], Wq1 [512,2048], Wq2 [2048,512],
  Wk1/Wv1 [256,2048], Wk2/Wv2 [512,256], Wo [2048,2048]
  HQ=16 q-heads, HKV=4 kv-heads (GROUP=4), DH=128.

Sharding: core c = (b, g) with b = c//4 (data-parallel over batch),
g = c%4 (tensor-parallel over head groups). Each core owns q-heads
{4g..4g+3} and kv-head g for its batch's 2048 tokens. Host folds
(W2_head @ W1) into per-head direct projections (exact math in float64),
pre-transposes x[b] to [D, T], and sums the 4 per-core partial Wo
outputs per batch.

Device per core:
  phase 1: qT [128, 4*2048], kT [128, 2048], vT [128, 2048] via folded
           weights; vT PE-transposed into v (key tokens on partitions),
           interleaved with the projection loop.
  phase 2: per (qc, h) chunk of 512 queries: scores^T = kT.T @ qT chunk,
           E = exp(scale*S) (ACT), PV accumulate + ones-row sumexp
           matmul, store unnormalized attnT (bf16), sumexp rows.
           Chunks are software-pipelined (next chunk's scores interleave
           with this chunk's PV) and ordered qc-major so phases 3/4 for
           qc can start while qc+1 is still in attention.
  phase 3: per qc: PE-transpose sumexp rows [4,128] -> [128,4], vector
           reciprocal -> rT (per-token-per-head normalizers).
  phase 4: per (token tile, dmodel chunk): 4 per-head Wo matmuls, then a
           fused scale-and-accumulate chain spread over scalar/vector/
           gpsimd engines, normalized output DMA'd to DRAM.
"""

import os
import numpy as np

import concourse.bass as bass
import concourse.tile as tile
from concourse import mybir
from concourse import bass_utils

D_MODEL, HQ, HKV, RQ, RKV = 2048, 16, 4, 512, 256
DH = D_MODEL // HQ            # 128
GROUP = HQ // HKV             # 4
B, T = 2, 2048
NCORES = 8
NGROUP = 4                    # tensor-parallel groups (one per kv head)
HPC = HQ // NGROUP            # 4 q-heads per core
SCALE = 1.0 / np.sqrt(DH)

NK = D_MODEL // 128           # 16 contraction tiles over D
NTC = T // 512                # 4 token chunks of 512
NKK = T // 128                # 16 key tiles of 128
NQC = T // 512                # 4 query chunks of 512
NTT = T // 128                # 16 token tiles of 128

f32 = mybir.dt.float32


class _TC(tile.TileContext):
    pass


_nop_ctr = [0]


def _split_multi_waits(nc):
    """This walrus build's CoreV3 lowering accepts only ONE sync-wait per
    instruction; move extra waits onto same-engine single-wait nops inserted
    immediately before the instruction."""
    for f in nc.m.functions:
        for bb in f.blocks:
            insts = list(bb.instructions)
            out = []
            changed = False
            for ins in insts:
                si = ins.sync_info
                if si is not None and si.on_wait and len(si.on_wait) > 1:
                    waits = list(si.on_wait)
                    for w in waits[:-1]:
                        _nop_ctr[0] += 1
                        nop = mybir.InstNoOp(
                            name=f"waitsplit_{_nop_ctr[0]}",
                            ins=[],
                            outs=[],
                            engine=ins.engine,
                        )
                        nop.sync_info = mybir.SyncInfo(on_wait=[w], on_update=[])
                        nc.register_instruction(nop)
                        out.append(nop)
                    ins.sync_info = mybir.SyncInfo(
                        on_wait=[waits[-1]], on_update=list(si.on_update)
                    )
                    changed = True
                out.append(ins)
            if changed:
                bb.instructions = out


def _build(mmdt, use_mask):
    nc = bass.Bass(trn_type="TRN2")
    xT = nc.dram_tensor("xT", (D_MODEL, T), mmdt, kind="ExternalInput")
    wq = nc.dram_tensor("wq", (D_MODEL, HPC * DH), mmdt, kind="ExternalInput")
    wk = nc.dram_tensor("wk", (D_MODEL, DH), mmdt, kind="ExternalInput")
    wv = nc.dram_tensor("wv", (D_MODEL, DH), mmdt, kind="ExternalInput")
    woT = nc.dram_tensor("woT", (HPC * DH, D_MODEL), mmdt, kind="ExternalInput")
    identm = nc.dram_tensor("identm", (128, 128), mmdt, kind="ExternalInput")
    identf = nc.dram_tensor("identf", (128, 128), f32, kind="ExternalInput")
    if use_mask:
        # pre-transposed, pre-scaled by sqrt(DH): [k, q]
        maskT = nc.dram_tensor("maskT", (T, T), f32, kind="ExternalInput")
    else:
        maskT = None
    out = nc.dram_tensor("out", (T, D_MODEL), f32, kind="ExternalOutput")

    Exp = mybir.ActivationFunctionType.Exp
    Copy = mybir.ActivationFunctionType.Copy
    Mult = mybir.AluOpType.mult
    Add = mybir.AluOpType.add

    with _TC(nc) as tc:
        with (
            tc.tile_pool(name="persist", bufs=1) as persist,
            tc.tile_pool(name="consts", bufs=1) as consts,
        ):
            qT_s = persist.tile([128, HPC * T], mmdt)     # head h at cols h*T
            kT_s = persist.tile([128, T], mmdt)
            vT_s = persist.tile([128, T], mmdt)
            v_s = persist.tile([128, T], mmdt)            # kk-tile t at cols t*128
            attnT_s = persist.tile([128, HPC * T], mmdt)  # unnormalized PV
            sumexp_s = persist.tile([128, T], f32)        # head h on partition 32*h
            rT_s = persist.tile([128, NTT * HPC], f32)    # recip, tok on partition
            woT_s = persist.tile([128, HPC * D_MODEL], mmdt)
            onesf_s = consts.tile([128, 1], f32)
            onesb_s = consts.tile([128, 1], mmdt)
            identm_s = consts.tile([128, 128], mmdt)
            identf_s = consts.tile([128, 128], f32)
            nc.vector.memset(onesf_s[:], 1.0)
            nc.vector.memset(onesb_s[:], 1.0)
            # unused partitions of sumexp_s flow through the phase-3
            # transpose; init so no garbage/non-finite values are read
            nc.vector.memset(sumexp_s[:], 1.0)

            # ---------------- phase 1: QKV projections ----------------
            # Weight/const DMA goes on the (otherwise idle) gpsimd queue;
            # sync queue carries x tiles. Per-kd slices so the first
            # matmuls can start early. Two passes (k/v then q) so every
            # psum tag is double-buffered within the 8-bank budget; x is
            # simply re-DMA'd for the second pass.
            with (
                tc.tile_pool(name="wgt", bufs=1) as wgt,
                tc.tile_pool(name="xin", bufs=6) as xin,
            ):
                wq_s = wgt.tile([128, NK * HPC * DH], mmdt)
                wk_s = wgt.tile([128, NK * DH], mmdt)
                wv_s = wgt.tile([128, NK * DH], mmdt)
                for kd in range(8):
                    ksl = slice(kd * 128, (kd + 1) * 128)
                    nc.gpsimd.dma_start(
                        wk_s[:, kd * 128 : (kd + 1) * 128], wk[ksl, :]
                    )
                    nc.gpsimd.dma_start(
                        wv_s[:, kd * 128 : (kd + 1) * 128], wv[ksl, :]
                    )
                    if kd == 3:
                        nc.gpsimd.dma_start(identm_s[:], identm[:])
                        nc.gpsimd.dma_start(identf_s[:], identf[:])
                for g4 in range(2, NK // 4):
                    gsl = slice(g4 * 512, (g4 + 1) * 512)
                    nc.gpsimd.dma_start(
                        wk_s[:, g4 * 512 : (g4 + 1) * 512].rearrange(
                            "p (t m) -> p t m", t=4
                        ),
                        wk[gsl, :].rearrange("(t p) m -> p t m", p=128),
                    )
                    nc.gpsimd.dma_start(
                        wv_s[:, g4 * 512 : (g4 + 1) * 512].rearrange(
                            "p (t m) -> p t m", t=4
                        ),
                        wv[gsl, :].rearrange("(t p) m -> p t m", p=128),
                    )
                for kd in range(NK):
                    nc.scalar.dma_start(
                        wq_s[:, kd * 512 : (kd + 1) * 512],
                        wq[kd * 128 : (kd + 1) * 128, :],
                    )
                for h in range(HPC):
                    nc.scalar.dma_start(
                        woT_s[:, h * D_MODEL : (h + 1) * D_MODEL],
                        woT[h * 128 : (h + 1) * 128, :],
                    )

                # single pass: all six projections accumulate concurrently
                # (6 psum banks), v transposed as each chunk's vT finishes
                with (
                    tc.tile_pool(name="qkvp", bufs=1, space="PSUM") as qkvp,
                    tc.tile_pool(name="trp", bufs=2, space="PSUM") as trp,
                ):
                    def emit_vtr(n):
                        for t in range(n * 4, n * 4 + 4):
                            tr = trp.tile([128, 128], mmdt, tag="tr", name=f"tr_{t}")
                            nc.tensor.transpose(
                                tr[:], vT_s[:, t * 128 : (t + 1) * 128], identm_s[:]
                            )
                            nc.vector.tensor_copy(
                                v_s[:, t * 128 : (t + 1) * 128], tr[:]
                            )

                    for n in range(NTC):
                        nsl = slice(n * 512, (n + 1) * 512)
                        ps_q = [
                            qkvp.tile([128, 512], f32, tag=f"psq{j}", name=f"psq{j}_{n}")
                            for j in range(HPC)
                        ]
                        ps_k = qkvp.tile([128, 512], f32, tag="psk", name=f"psk_{n}")
                        ps_v = qkvp.tile([128, 512], f32, tag="psv", name=f"psv_{n}")
                        for kd in range(NK):
                            xt = xin.tile([128, 512], mmdt, tag="xt", name=f"xt_{n}_{kd}")
                            nc.sync.dma_start(
                                xt[:], xT[kd * 128 : (kd + 1) * 128, nsl]
                            )
                            st, sp = kd == 0, kd == NK - 1
                            for j in range(HPC):
                                nc.tensor.matmul(
                                    ps_q[j][:],
                                    wq_s[:, kd * 512 + j * 128 : kd * 512 + (j + 1) * 128],
                                    xt[:],
                                    start=st, stop=sp,
                                )
                            nc.tensor.matmul(
                                ps_k[:], wk_s[:, kd * 128 : (kd + 1) * 128], xt[:],
                                start=st, stop=sp,
                            )
                            nc.tensor.matmul(
                                ps_v[:], wv_s[:, kd * 128 : (kd + 1) * 128], xt[:],
                                start=st, stop=sp,
                            )
                        nc.scalar.activation(qT_s[:, 0 * T + n * 512 : 0 * T + (n + 1) * 512], ps_q[0][:], Copy)
                        nc.scalar.activation(qT_s[:, 1 * T + n * 512 : 1 * T + (n + 1) * 512], ps_q[1][:], Copy)
                        nc.vector.tensor_copy(qT_s[:, 2 * T + n * 512 : 2 * T + (n + 1) * 512], ps_q[2][:])
                        nc.vector.tensor_copy(qT_s[:, 3 * T + n * 512 : 3 * T + (n + 1) * 512], ps_q[3][:])
                        nc.vector.tensor_copy(kT_s[:, nsl], ps_k[:])
                        nc.vector.tensor_copy(vT_s[:, nsl], ps_v[:])
                        if n > 0:
                            emit_vtr(n - 1)
                    emit_vtr(NTC - 1)

            # ---------------- phases 2+3+4 interleaved ----------------
            with (
                tc.tile_pool(name="epool", bufs=36) as epool,
                tc.tile_pool(name="esum", bufs=2) as esump,
                tc.tile_pool(name="omg", bufs=4) as omg,
                tc.tile_pool(name="anorm", bufs=6) as anorm,
                tc.tile_pool(name="mpool", bufs=3) as mpool,
                tc.tile_pool(name="stp", bufs=3, space="PSUM") as stp,
                tc.tile_pool(name="pvp", bufs=1, space="PSUM") as pvp,
                tc.tile_pool(name="sump", bufs=1, space="PSUM") as sump,
                tc.tile_pool(name="ntrp", bufs=2, space="PSUM") as ntrp,
                tc.tile_pool(name="wops", bufs=1, space="PSUM") as wops,
            ):
                chunks = [(qc, h) for qc in range(NQC) for h in range(HPC)]

                def emit_scores(ci, kt):
                    qc, h = chunks[ci]
                    qsl = qT_s[:, h * T + qc * 512 : h * T + (qc + 1) * 512]
                    ps_st = stp.tile([128, 512], f32, tag="st", name=f"st_{ci}_{kt}")
                    nc.tensor.matmul(
                        ps_st[:],
                        kT_s[:, kt * 128 : (kt + 1) * 128],
                        qsl,
                        start=True, stop=True,
                    )
                    if use_mask:
                        mt = mpool.tile([128, 512], f32, tag="mt", name=f"mt_{ci}_{kt}")
                        nc.sync.dma_start(
                            mt[:],
                            maskT[kt * 128 : (kt + 1) * 128, qc * 512 : (qc + 1) * 512],
                        )
                        nc.vector.tensor_add(ps_st[:], ps_st[:], mt[:])
                    e = epool.tile([128, 512], mmdt, tag="e", name=f"e_{ci}_{kt}")
                    nc.scalar.activation(e[:], ps_st[:], Exp, scale=SCALE)
                    return e

                aN_store = {}

                def emit_normA(h0, h1, it):
                    # stage 1 of in-place attnT normalization: transpose two
                    # head tiles -> tokens on partitions, scale by recip
                    # (per-partition). Stage 2 (normB) runs in a later drain
                    # slot so PE never waits on the vector scale.
                    for hh in (h0, h1):
                        asl = attnT_s[:, hh * T + it * 128 : hh * T + (it + 1) * 128]
                        tr1 = ntrp.tile([128, 128], mmdt, tag="ntr", name=f"tr1_{hh}_{it}")
                        nc.tensor.transpose(tr1[:], asl, identm_s[:])
                        aN = anorm.tile([128, 128], mmdt, tag="an", name=f"aN_{hh}_{it}")
                        nc.vector.tensor_scalar_mul(
                            aN[:], tr1[:], rT_s[:, it * HPC + hh : it * HPC + hh + 1]
                        )
                        aN_store[(hh, it)] = aN

                def emit_normB(h0, h1, it):
                    # stage 2: transpose the scaled tiles back into attnT
                    for hh in (h0, h1):
                        aN = aN_store.pop((hh, it))
                        tr2 = ntrp.tile([128, 128], mmdt, tag="ntr", name=f"tr2_{hh}_{it}")
                        nc.tensor.transpose(tr2[:], aN[:], identm_s[:])
                        nc.vector.tensor_copy(
                            attnT_s[:, hh * T + it * 128 : hh * T + (it + 1) * 128],
                            tr2[:],
                        )

                def emit_p4_item(it, dc, drain_eng, pool=None, tag="wo"):
                    # 4 per-head Wo matmuls accumulate into one psum bank
                    p = (pool or wops).tile(
                        [128, 512], f32, tag=tag, name=f"wo_{it}_{dc}"
                    )
                    for h in range(HPC):
                        nc.tensor.matmul(
                            p[:],
                            attnT_s[:, h * T + it * 128 : h * T + (it + 1) * 128],
                            woT_s[:, h * D_MODEL + dc * 512 : h * D_MODEL + (dc + 1) * 512],
                            start=(h == 0), stop=(h == HPC - 1),
                        )
                    oo = omg.tile([128, 512], f32, tag="oo", name=f"oo_{it}_{dc}")
                    if drain_eng == "scalar":
                        nc.scalar.activation(oo[:], p[:], Copy)
                    else:
                        nc.vector.tensor_copy(oo[:], p[:])
                    nc.sync.dma_start(
                        out[it * 128 : (it + 1) * 128, dc * 512 : (dc + 1) * 512],
                        oo[:],
                    )

                def drain_unit(u):
                    kind = u[0]
                    if kind == "normA":
                        emit_normA(u[1], u[2], u[3])
                    elif kind == "normB":
                        emit_normB(u[1], u[2], u[3])
                    else:
                        emit_p4_item(u[1], u[2], u[3])

                p4q = []  # deferred norm/p4 work units
                es_cur = [emit_scores(0, kt) for kt in range(NKK)]
                for ci in range(len(chunks)):
                    qc, h = chunks[ci]
                    ps_pv = pvp.tile([128, 512], f32, tag="pv", name=f"pv_{ci}")
                    # sumexp: kt 0-5 summed on PE (ones-matmuls), kt 6-8 on
                    # gpsimd, kt 9-15 on vector; two folding matmuls at end.
                    ps_sum = sump.tile([1, 512], f32, tag="sum", name=f"sum_{ci}")
                    esumG = esump.tile([128, 512], f32, tag="esG", name=f"esG_{ci}")
                    esumV = esump.tile([128, 512], f32, tag="esV", name=f"esV_{ci}")
                    es_next = []
                    for kt in range(NKK):
                        st, sp = kt == 0, kt == NKK - 1
                        nc.tensor.matmul(
                            ps_pv[:],
                            v_s[:, kt * 128 : (kt + 1) * 128],
                            es_cur[kt][:],
                            start=st, stop=sp,
                        )
                        if kt >= 12:
                            # PE ones-matmuls last: the chunk end depends only
                            # on fresh PV output, never on lagging engines
                            nc.tensor.matmul(
                                ps_sum[:], onesb_s[:], es_cur[kt][:],
                                start=(kt == 12), stop=False,
                            )
                        elif kt == 1:
                            nc.gpsimd.tensor_add(esumG[:], es_cur[0][:], es_cur[1][:])
                        elif 2 <= kt <= 4:
                            nc.gpsimd.tensor_add(esumG[:], esumG[:], es_cur[kt][:])
                        elif kt == 6:
                            nc.vector.tensor_add(esumV[:], es_cur[5][:], es_cur[6][:])
                        elif kt >= 7:
                            nc.vector.tensor_add(esumV[:], esumV[:], es_cur[kt][:])
                        if sp:
                            # free the pv bank as soon as accumulation stops
                            nc.vector.tensor_copy(
                                attnT_s[:, h * T + qc * 512 : h * T + (qc + 1) * 512],
                                ps_pv[:],
                            )
                        if ci + 1 < len(chunks):
                            es_next.append(emit_scores(ci + 1, kt))
                        if p4q and kt % 2 == 1:
                            drain_unit(p4q.pop(0))
                    # fold the gpsimd/vector partial sums into ps_sum
                    nc.tensor.matmul(
                        ps_sum[:], onesf_s[:], esumG[:], start=False, stop=False,
                    )
                    nc.tensor.matmul(
                        ps_sum[:], onesf_s[:], esumV[:], start=False, stop=True,
                    )
                    nc.scalar.activation(
                        sumexp_s[32 * h : 32 * h + 1, qc * 512 : (qc + 1) * 512],
                        ps_sum[0:1, :],
                        Copy,
                    )
                    es_cur = es_next
                    if h == HPC - 1:
                        # phase 3 for this qc: transpose sumexp rows, recip.
                        # Heads live at partitions {0,32,64,96} (engine
                        # partition-start constraint), so transpose the full
                        # 128 partitions and recip the 4 head columns.
                        for tt in range(4):
                            it = qc * 4 + tt
                            ps_r = ntrp.tile([128, 128], f32, tag="ntr", name=f"psr_{it}")
                            nc.tensor.transpose(
                                ps_r[:],
                                sumexp_s[:, qc * 512 + tt * 128 : qc * 512 + (tt + 1) * 128],
                                identf_s[:],
                            )
                            for hh in range(HPC):
                                nc.vector.reciprocal(
                                    rT_s[:, it * HPC + hh : it * HPC + hh + 1],
                                    ps_r[:, 32 * hh : 32 * hh + 1],
                                )
                        if qc < NQC - 1:
                            for tt in range(4):
                                it = qc * 4 + tt
                                p4q.append(("normA", 0, 1, it))
                                p4q.append(("normA", 2, 3, it))
                                p4q.append(("normB", 0, 1, it))
                                p4q.append(("normB", 2, 3, it))
                                for dc in range(4):
                                    p4q.append(("p4", it, dc, "vector"))
                        else:
                            # tail qc: pipeline across token tiles so p4
                            # items always have independent norm work of the
                            # next tile between dependent stages
                            base = qc * 4
                            p4q.append(("normA", 0, 1, base))
                            p4q.append(("normA", 2, 3, base))
                            p4q.append(("normB", 0, 1, base))
                            p4q.append(("normB", 2, 3, base))
                            for tt in range(1, 5):
                                if tt < 4:
                                    it = base + tt
                                    p4q.append(("normA", 0, 1, it))
                                    p4q.append(("normA", 2, 3, it))
                                for dc in range(4):
                                    p4q.append(("p4", base + tt - 1, dc, "vector"))
                                if tt < 4:
                                    it = base + tt
                                    p4q.append(("normB", 0, 1, it))
                                    p4q.append(("normB", 2, 3, it))
                # tail: scalar is exp-free and the score/pv/sum banks are
                # done, so alternate drain engines and rotate p4 psums over
                # three banks to pipeline back-to-back items
                tail_psum = [(wops, "wo"), (sump, "sum"), (pvp, "pv")]
                n_p4 = 0
                for u in p4q:
                    if u[0] == "p4":
                        pool, tag = tail_psum[n_p4 % 3]
                        emit_p4_item(
                            u[1], u[2],
                            "scalar" if n_p4 % 2 else "vector",
                            pool=pool, tag=tag,
                        )
                        n_p4 += 1
                    else:
                        drain_unit(u)

    _split_multi_waits(nc)
    return nc


_cache = {}


def _get_nc(mmdt_name, use_mask):
    key = (mmdt_name, use_mask)
    if key not in _cache:
        _cache[key] = _build(getattr(mybir.dt, mmdt_name), use_mask)
    return _cache[key]


def _np_dt(mmdt_name):
    if mmdt_name == "bfloat16":
        import ml_dtypes

        return ml_dtypes.bfloat16
    return np.float32


def _prep_inputs(x, attn_mask, Wq1, Wq2, Wk1, Wk2, Wv1, Wv2, Wo, mmdt_name):
    ndt = _np_dt(mmdt_name)
    identm = np.eye(128, dtype=np.float32).astype(ndt)
    identf = np.eye(128, dtype=np.float32)
    use_mask = bool(np.any(attn_mask))
    maskT = None
    if use_mask:
        maskT = np.ascontiguousarray(attn_mask[0, 0].T * np.sqrt(DH)).astype(
            np.float32
        )
    Wq1_64, Wq2_64 = Wq1.astype(np.float64), Wq2.astype(np.float64)
    Wk1_64, Wk2_64 = Wk1.astype(np.float64), Wk2.astype(np.float64)
    Wv1_64, Wv2_64 = Wv1.astype(np.float64), Wv2.astype(np.float64)
    xT_b = [
        np.ascontiguousarray(np.asarray(x[b]).T).astype(ndt) for b in range(B)
    ]
    in_maps = []
    for c in range(NCORES):
        b, g = divmod(c, NGROUP)
        h0 = g * HPC
        wq_f = (Wq2_64[h0 * DH : (h0 + HPC) * DH] @ Wq1_64).T  # [D, HPC*DH]
        wk_f = (Wk2_64[g * DH : (g + 1) * DH] @ Wk1_64).T      # [D, DH]
        wv_f = (Wv2_64[g * DH : (g + 1) * DH] @ Wv1_64).T
        woT_c = np.ascontiguousarray(Wo[:, h0 * DH : (h0 + HPC) * DH].T)
        m = {
            "xT": xT_b[b],
            "wq": np.ascontiguousarray(wq_f).astype(ndt),
            "wk": np.ascontiguousarray(wk_f).astype(ndt),
            "wv": np.ascontiguousarray(wv_f).astype(ndt),
            "woT": woT_c.astype(ndt),
            "identm": identm,
            "identf": identf,
        }
        if use_mask:
            m["maskT"] = maskT
        in_maps.append(m)
    return in_maps, use_mask


def run(x, attn_mask, Wq1, Wq2, Wk1, Wk2, Wv1, Wv2, Wo, **spmd_kwargs):
    mmdt_name = os.environ.get("BASS_MLA_DT", "bfloat16")
    in_maps, use_mask = _prep_inputs(
        x, attn_mask, Wq1, Wq2, Wk1, Wk2, Wv1, Wv2, Wo, mmdt_name
    )
    nc = _get_nc(mmdt_name, use_mask)
    res = bass_utils.run_bass_kernel_spmd(
        nc, in_maps, core_ids=list(range(NCORES)), **spmd_kwargs
    )
    out = np.zeros((B, T, D_MODEL), np.float64)
    for c in range(NCORES):
        out[c // NGROUP] += res.results[c]["out"]
    return out.astype(np.float32), res


def kernel(x, attn_mask, Wq1, Wq2, Wk1, Wk2, Wv1, Wv2, Wo):
    out, _ = run(x, attn_mask, Wq1, Wq2, Wk1, Wk2, Wv1, Wv2, Wo)
    return out
